# revision 16
# baseline (speedup 1.0000x reference)
"""MoE (noisy top-k gating, E=8 experts, K=4) forward on 8 trn2 NeuronCores.

Sharding: expert-parallel with capacity-based token gathering.

Phase 1 (device, token-parallel): each core computes the noisy-top-k gating
for B/8 tokens (fp32 matmuls + Max8 sort + Exp/Erf activations) and returns
dense gates [B/8, E] and the top-k inclusion probabilities (for the
load-balance loss).

Host routing: from the device-computed gates, build each expert's token
list (~B*K/E tokens), pad to CAP, gather the bf16 token vectors.

Phase 2 (device, expert-parallel): core e runs the dense 2-layer MLP for
its expert over its gathered CAP tokens in bf16 (transposed-activation
dataflow xT -> hT -> oT), scales by the gathered gate row, and returns the
partial oT [O, CAP]. The host scatter-adds the 8 partials into y (the
expert-combine reduction) and finishes the scalar loss.

If any expert is assigned more than CAP tokens (never for the benchmark
shapes: observed max 2101 vs CAP 2304), a dense fallback kernel computes
all 4096 tokens on every expert.
"""

import numpy as np
import ml_dtypes

B, D, H, O, E, K = 4096, 1024, 4096, 1024, 8, 4
NOISE_EPS = 0.01
LOSS_COEF = 0.01

P = 128          # partitions
TB = B // P      # 32 token tiles of 128
KC = D // P      # 8 contraction chunks for D
HC = H // P      # 32 h tiles
OC = O // P      # 8 o tiles

NCORE = 8
TBL = TB // NCORE  # 4 token tiles per core in phase 1

CAP = 2176       # per-expert token capacity (margin over observed max 2101)
# large chunks first: chunk-0's layer-1 span covers the w2 prefetch DMA
CHUNKS = [(0, 512), (512, 512), (1024, 512), (1536, 384), (1920, 256)]

MC = 512         # dense-fallback chunk
NM = B // MC

_CACHE = {}


def _patch_act_tables():
    """Steer Exp/Ln to the combined natural_log_exp table set.

    The act-table chooser greedily picks the first set containing each
    function, bouncing exp_and_others -> natural_log -> exp_and_others ->
    sigmoid (4 x ~2.7us loads) for our Exp,Ln,Exp,Erf sequence. Hiding Exp/Ln
    from the single-function sets makes it settle on natural_log_exp (2
    loads). Only the choice changes; set ids still index act_info.json.
    """
    import concourse.bacc as bacc
    import concourse.mybir as mybir

    if getattr(bacc, "_moe_act_patched", False):
        return
    orig = bacc.get_activation_tables
    AF = mybir.ActivationFunctionType

    def patched(arch):
        t = {k: set(v) for k, v in orig(arch).items()}
        if "natural_log_exp_and_others" in t:
            t.get("exp_and_others", set()).discard(AF.Exp)
            t.get("natural_log", set()).discard(AF.Ln)
        return t

    bacc.get_activation_tables = patched
    bacc._moe_act_patched = True


def _emit_gating(nc, tc, bass, mybir, tbl, xg_d, wg_sb, nz_sb, pools):
    """Gating math for tbl token tiles. Returns (gates_all, probs_sb) sbuf APs."""
    f32 = mybir.dt.float32
    AF = mybir.ActivationFunctionType
    ALU = mybir.AluOpType
    AX = mybir.AxisListType
    pers, xgp, pg = pools

    raw_all = pers.tile([P, tbl, 2 * E], f32, tag="raw")
    sp_all = pers.tile([P, tbl, E], f32, tag="sp")
    noisy_all = pers.tile([P, tbl, E], f32, tag="noisy")
    sort_all = pers.tile([P, tbl, E], f32, tag="sort")
    diff_all = pers.tile([P, tbl, E], f32, tag="diff")
    expd_all = pers.tile([P, tbl, E], f32, tag="expd")
    mask_all = pers.tile([P, tbl, E], f32, tag="mask")
    gme_all = pers.tile([P, tbl, E], f32, tag="gme")
    gs_all = pers.tile([P, tbl], f32, tag="gs")
    rs_all = pers.tile([P, tbl], f32, tag="rs")
    zin_all = pers.tile([P, tbl, E], f32, tag="zin")
    zout_all = pers.tile([P, tbl, E], f32, tag="zout")
    rstd_all = pers.tile([P, tbl, E], f32, tag="rstd")
    min_all = pers.tile([P, tbl, E], mybir.dt.uint32, tag="min")
    pin_all = pers.tile([P, tbl, E], f32, tag="pin")
    probs_sb = pers.tile([P, tbl, E], f32, tag="probs")
    gates_all = pers.tile([P, tbl, E], f32, tag="gates")

    clean_all = raw_all[:, :, 0:E]

    for t in range(tbl):
        xg_t = xgp.tile([P, KC, P], f32, tag="xg")
        nc.sync.dma_start(xg_t[:], xg_d.ap()[t])
        ps = pg.tile([P, 2 * E], f32, tag="pg")
        for c in range(KC):
            nc.tensor.matmul(
                ps[:], xg_t[:, c, :], wg_sb[:, c, :],
                start=(c == 0), stop=(c == KC - 1),
            )
        nc.vector.tensor_copy(raw_all[:, t, :], ps[:])

    # stddev = softplus(rawnoise) + eps = ln(1 + exp(r)) + eps
    nc.scalar.activation(sp_all[:], raw_all[:, :, E : 2 * E], AF.Exp)
    nc.scalar.activation(sp_all[:], sp_all[:], AF.Ln, bias=1.0)
    nc.vector.tensor_scalar_add(sp_all[:], sp_all[:], NOISE_EPS)
    # noisy = clean + noise * stddev
    nc.vector.tensor_tensor(noisy_all[:], nz_sb[:], sp_all[:], ALU.mult)
    nc.vector.tensor_tensor(noisy_all[:], noisy_all[:], clean_all, ALU.add)
    nc.vector.reciprocal(rstd_all[:], sp_all[:])
    for t in range(tbl):
        nc.vector.max(sort_all[:, t, :], noisy_all[:, t, :])
    for t in range(tbl):
        thr4 = sort_all[:, t, K - 1 : K]
        thr5 = sort_all[:, t, K : K + 1]
        nc.vector.tensor_scalar(
            diff_all[:, t, :], noisy_all[:, t, :], thr4, None, op0=ALU.subtract
        )
        nc.vector.tensor_scalar(
            zin_all[:, t, :], clean_all[:, t, :], thr5, None, op0=ALU.subtract
        )
        nc.vector.tensor_scalar(
            zout_all[:, t, :], clean_all[:, t, :], thr4, None, op0=ALU.subtract
        )
        nc.vector.tensor_scalar(
            min_all[:, t, :], noisy_all[:, t, :], thr5, None, op0=ALU.is_gt
        )
    nc.scalar.activation(expd_all[:], diff_all[:], AF.Exp)
    nc.vector.tensor_scalar(mask_all[:], diff_all[:], 0.0, None, op0=ALU.is_ge)
    nc.vector.tensor_tensor(gme_all[:], expd_all[:], mask_all[:], ALU.mult)
    nc.vector.tensor_reduce(gs_all[:], gme_all[:], axis=AX.X, op=ALU.add)
    nc.vector.reciprocal(rs_all[:], gs_all[:])
    rsb = rs_all[:, :, None].to_broadcast([P, tbl, E])
    nc.vector.tensor_tensor(gates_all[:], gme_all[:], rsb, ALU.mult)
    # prob = Phi(z) = 0.5 * erf(z / sqrt(2)) + 0.5
    nc.vector.tensor_tensor(zin_all[:], zin_all[:], rstd_all[:], ALU.mult)
    nc.vector.tensor_tensor(zout_all[:], zout_all[:], rstd_all[:], ALU.mult)
    isq2 = float(1.0 / np.sqrt(2.0))
    nc.scalar.activation(pin_all[:], zin_all[:], AF.Erf, scale=isq2)
    nc.scalar.activation(probs_sb[:], zout_all[:], AF.Erf, scale=isq2)
    nc.vector.tensor_scalar(
        pin_all[:], pin_all[:], 0.5, 0.5, op0=ALU.mult, op1=ALU.add
    )
    nc.vector.tensor_scalar(
        probs_sb[:], probs_sb[:], 0.5, 0.5, op0=ALU.mult, op1=ALU.add
    )
    nc.vector.copy_predicated(probs_sb[:], min_all[:], pin_all[:])
    return gates_all, probs_sb


def _build_gate():
    """Phase-1: token-parallel gating; each core handles B/8 tokens."""
    import concourse.bacc as bacc
    import concourse.bass as bass
    import concourse.mybir as mybir
    import concourse.tile as tile

    _patch_act_tables()
    f32 = mybir.dt.float32
    nc = bacc.Bacc("TRN2", target_bir_lowering=False, debug=False)

    xg_d = nc.dram_tensor("xg", [TBL, P, KC, P], f32, kind="ExternalInput")
    wg_d = nc.dram_tensor("wg", [P, KC, 2 * E], f32, kind="ExternalInput")
    nz_d = nc.dram_tensor("nz", [P, TBL, E], f32, kind="ExternalInput")
    gates_d = nc.dram_tensor("gates", [P, TBL * E], f32, kind="ExternalOutput")
    probs_d = nc.dram_tensor("probs", [P, TBL * E], f32, kind="ExternalOutput")

    with tile.TileContext(nc) as tc:
        with (
            tc.tile_pool(name="persist", bufs=1) as pers,
            tc.tile_pool(name="xgp", bufs=3) as xgp,
            tc.tile_pool(name="pg", bufs=2, space="PSUM") as pg,
        ):
            wg_sb = pers.tile([P, KC, 2 * E], f32, tag="wg")
            nc.sync.dma_start(wg_sb[:], wg_d.ap())
            nz_sb = pers.tile([P, TBL, E], f32, tag="nz")
            nc.sync.dma_start(nz_sb[:], nz_d.ap())
            gates_all, probs_sb = _emit_gating(
                nc, tc, bass, mybir, TBL, xg_d, wg_sb, nz_sb, (pers, xgp, pg)
            )
            nc.sync.dma_start(gates_d.ap(), gates_all.rearrange("p t e -> p (t e)"))
            nc.sync.dma_start(probs_d.ap(), probs_sb.rearrange("p t e -> p (t e)"))
    nc.compile()
    return nc


def _build_mlp():
    """Phase-2: expert-parallel MLP over CAP gathered tokens per core."""
    import concourse.bacc as bacc
    import concourse.bass as bass
    import concourse.mybir as mybir
    import concourse.tile as tile

    f32 = mybir.dt.float32
    bf16 = mybir.dt.bfloat16
    AF = mybir.ActivationFunctionType
    ALU = mybir.AluOpType

    _patch_act_tables()
    nc = bacc.Bacc("TRN2", target_bir_lowering=False, debug=False)

    xbt_d = nc.dram_tensor("xbt", [P, KC, CAP], bf16, kind="ExternalInput")
    w1_d = nc.dram_tensor("w1", [HC, P, D], bf16, kind="ExternalInput")
    w2_d = nc.dram_tensor("w2", [P, HC, O], bf16, kind="ExternalInput")
    b1_d = nc.dram_tensor("b1", [P, HC], f32, kind="ExternalInput")
    b2_d = nc.dram_tensor("b2", [P, OC], f32, kind="ExternalInput")
    g_d = nc.dram_tensor("g", [1, CAP], f32, kind="ExternalInput")
    oT_d = nc.dram_tensor("oT", [O, CAP], f32, kind="ExternalOutput")

    with tile.TileContext(nc) as tc:
        with (
            tc.tile_pool(name="persist", bufs=1) as pers,
            tc.tile_pool(name="xbp", bufs=2) as xbp,
            tc.tile_pool(name="w1p", bufs=4) as w1p,
            tc.tile_pool(name="hp", bufs=36) as hp,
            tc.tile_pool(name="op", bufs=4) as op_,
            tc.tile_pool(name="ph", bufs=2, space="PSUM") as ph,
            tc.tile_pool(name="po", bufs=2, space="PSUM") as po,
        ):
            b1_sb = pers.tile([P, HC], f32, tag="b1")
            nc.sync.dma_start(b1_sb[:], b1_d.ap())
            b2_sb = pers.tile([P, OC], f32, tag="b2")
            nc.sync.dma_start(b2_sb[:], b2_d.ap())
            gbc = pers.tile([P, CAP], f32, tag="gbc")
            nc.gpsimd.dma_start(
                out=gbc[:], in_=bass.AP(tensor=g_d, offset=0, ap=[[0, P], [1, CAP]])
            )
            # w2 is DMA'd in quarters, emitted after the first chunk's layer-1
            # so the startup DMAs that gate the first matmuls go first.
            w2_sb = pers.tile([P, HC, O], bf16, tag="w2")
            w2_started = False

            for ci, (start, sz) in enumerate(CHUNKS):
                xb_t = xbp.tile([P, KC, 512], bf16, tag="xb", name="xb_t")[:, :, :sz]
                # issue order drives the DMA queue: the first matmul's inputs
                # (w1 tile 0 first half, xb k-slice 0) must be issued first
                w1_0 = w1p.tile([P, D], bf16, tag="w1", name="w1_0")
                nc.sync.dma_start(w1_0[:, : D // 2], w1_d.ap()[0][:, : D // 2])
                nc.sync.dma_start(xb_t[:, 0, :], xbt_d.ap()[:, 0, start : start + sz])
                nc.sync.dma_start(w1_0[:, D // 2 :], w1_d.ap()[0][:, D // 2 :])
                for c in range(1, KC):
                    nc.sync.dma_start(
                        xb_t[:, c, :], xbt_d.ap()[:, c, start : start + sz]
                    )
                hts = []
                for i in range(HC):
                    if i == 0:
                        w1_t = w1_0
                    else:
                        w1_t = w1p.tile([P, D], bf16, tag="w1")
                        nc.sync.dma_start(
                            w1_t[:, : D // 2], w1_d.ap()[i][:, : D // 2]
                        )
                        nc.sync.dma_start(
                            w1_t[:, D // 2 :], w1_d.ap()[i][:, D // 2 :]
                        )
                    ps_h = ph.tile([P, 512], f32, tag="ph", name="ps_h")[:, :sz]
                    for c in range(KC):
                        nc.tensor.matmul(
                            ps_h[:],
                            w1_t[:, c * P : (c + 1) * P],
                            xb_t[:, c, :],
                            start=(c == 0),
                            stop=(c == KC - 1),
                        )
                    h_t = hp.tile([P, 512], bf16, tag="h", name="h_t")[:, :sz]
                    nc.scalar.activation(
                        h_t[:], ps_h[:], AF.Relu, bias=b1_sb[:, i : i + 1]
                    )
                    hts.append(h_t)
                if not w2_started:
                    w2_started = True
                    q = HC // 4
                    for qi in range(4):
                        nc.sync.dma_start(
                            w2_sb[:, qi * q : (qi + 1) * q, :],
                            w2_d.ap()[:, qi * q : (qi + 1) * q, :],
                        )
                for j in range(OC):
                    ps_o = po.tile([P, 512], f32, tag="po", name="ps_o")[:, :sz]
                    for i in range(HC):
                        nc.tensor.matmul(
                            ps_o[:],
                            w2_sb[:, i, j * P : (j + 1) * P],
                            hts[i][:],
                            start=(i == 0),
                            stop=(i == HC - 1),
                        )
                    o_t = op_.tile([P, 512], f32, tag="o", name="o_t")[:, :sz]
                    nc.scalar.activation(
                        o_t[:], ps_o[:], AF.Identity, bias=b2_sb[:, j : j + 1]
                    )
                    nc.vector.tensor_tensor(
                        o_t[:], o_t[:], gbc[:, start : start + sz], ALU.mult
                    )
                    nc.sync.dma_start(
                        oT_d.ap()[j * P : (j + 1) * P, start : start + sz], o_t[:]
                    )
    nc.compile()
    return nc


def _build_dense():
    """Fallback: every core computes its expert densely on all B tokens."""
    import concourse.bacc as bacc
    import concourse.bass as bass
    import concourse.mybir as mybir
    import concourse.tile as tile
    from concourse.masks import make_identity

    f32 = mybir.dt.float32
    bf16 = mybir.dt.bfloat16
    AF = mybir.ActivationFunctionType
    ALU = mybir.AluOpType
    AX = mybir.AxisListType

    _patch_act_tables()
    nc = bacc.Bacc("TRN2", target_bir_lowering=False, debug=False)

    xg_d = nc.dram_tensor("xg", [TB, P, KC, P], f32, kind="ExternalInput")
    xb_d = nc.dram_tensor("xb", [NM, P, KC, MC], bf16, kind="ExternalInput")
    w1_d = nc.dram_tensor("w1", [HC, P, D], bf16, kind="ExternalInput")
    w2_d = nc.dram_tensor("w2", [P, HC, O], bf16, kind="ExternalInput")
    b1_d = nc.dram_tensor("b1", [P, HC], f32, kind="ExternalInput")
    b2_d = nc.dram_tensor("b2", [P, OC], f32, kind="ExternalInput")
    wg_d = nc.dram_tensor("wg", [P, KC, 2 * E], f32, kind="ExternalInput")
    nz_d = nc.dram_tensor("nz", [P, TB, E], f32, kind="ExternalInput")
    sel_d = nc.dram_tensor("sel", [1, E], f32, kind="ExternalInput")
    oT_d = nc.dram_tensor("oT", [O, B], f32, kind="ExternalOutput")
    probs_d = nc.dram_tensor("probs", [P, TB * E], f32, kind="ExternalOutput")
    grow_d = nc.dram_tensor("grow", [TB, P], f32)

    with tile.TileContext(nc) as tc:
        with (
            tc.tile_pool(name="persist", bufs=1) as pers,
            tc.tile_pool(name="xgp", bufs=3) as xgp,
            tc.tile_pool(name="xbp", bufs=2) as xbp,
            tc.tile_pool(name="w1p", bufs=4) as w1p,
            tc.tile_pool(name="hp", bufs=36) as hp,
            tc.tile_pool(name="op", bufs=4) as op_,
            tc.tile_pool(name="pg", bufs=2, space="PSUM") as pg,
            tc.tile_pool(name="ptr", bufs=1, space="PSUM") as ptr,
            tc.tile_pool(name="ph", bufs=2, space="PSUM") as ph,
            tc.tile_pool(name="po", bufs=2, space="PSUM") as po,
        ):
            b1_sb = pers.tile([P, HC], f32, tag="b1")
            nc.sync.dma_start(b1_sb[:], b1_d.ap())
            b2_sb = pers.tile([P, OC], f32, tag="b2")
            nc.sync.dma_start(b2_sb[:], b2_d.ap())
            wg_sb = pers.tile([P, KC, 2 * E], f32, tag="wg")
            nc.sync.dma_start(wg_sb[:], wg_d.ap())
            nz_sb = pers.tile([P, TB, E], f32, tag="nz")
            nc.sync.dma_start(nz_sb[:], nz_d.ap())
            sel_sb = pers.tile([P, E], f32, tag="sel")
            nc.gpsimd.dma_start(
                out=sel_sb[:],
                in_=bass.AP(tensor=sel_d, offset=0, ap=[[0, P], [1, E]]),
            )
            ident = pers.tile([P, P], f32, tag="ident")
            make_identity(nc, ident)
            w2_sb = pers.tile([P, HC, O], bf16, tag="w2")
            nc.sync.dma_start(w2_sb[:], w2_d.ap())
            gcol = pers.tile([P, TB], f32, tag="gcol")
            gT_sb = pers.tile([TB, P], f32, tag="gT")
            gbc = pers.tile([P, B], f32, tag="gbc")

            gates_all, probs_sb = _emit_gating(
                nc, tc, bass, mybir, TB, xg_d, wg_sb, nz_sb, (pers, xgp, pg)
            )
            nc.sync.dma_start(probs_d.ap(), probs_sb.rearrange("p t e -> p (t e)"))
            # own-expert gate column: dot(gates, sel)
            gsel = pers.tile([P, TB, E], f32, tag="gsel")
            selb = sel_sb[:, None, :].to_broadcast([P, TB, E])
            nc.vector.tensor_tensor(gsel[:], gates_all[:], selb, ALU.mult)
            nc.vector.tensor_reduce(gcol[:], gsel[:], axis=AX.X, op=ALU.add)
            # broadcast gcol across partitions via transpose + DRAM round-trip
            ps_tr = ptr.tile([TB, P], f32, tag="ptr")
            nc.tensor.transpose(ps_tr[:], gcol[:], ident[:])
            nc.vector.tensor_copy(gT_sb[:], ps_tr[:])
            nc.sync.dma_start(grow_d.ap(), gT_sb[:])
            nc.gpsimd.dma_start(
                out=gbc[:], in_=bass.AP(tensor=grow_d, offset=0, ap=[[0, P], [1, B]])
            )

            for m in range(NM):
                xb_t = xbp.tile([P, KC, MC], bf16, tag="xb")
                nc.sync.dma_start(xb_t[:], xb_d.ap()[m])
                hts = []
                for i in range(HC):
                    w1_t = w1p.tile([P, D], bf16, tag="w1")
                    nc.sync.dma_start(w1_t[:], w1_d.ap()[i])
                    ps_h = ph.tile([P, MC], f32, tag="ph")
                    for c in range(KC):
                        nc.tensor.matmul(
                            ps_h[:],
                            w1_t[:, c * P : (c + 1) * P],
                            xb_t[:, c, :],
                            start=(c == 0),
                            stop=(c == KC - 1),
                        )
                    h_t = hp.tile([P, MC], bf16, tag="h")
                    nc.scalar.activation(
                        h_t[:], ps_h[:], AF.Relu, bias=b1_sb[:, i : i + 1]
                    )
                    hts.append(h_t)
                for j in range(OC):
                    ps_o = po.tile([P, MC], f32, tag="po")
                    for i in range(HC):
                        nc.tensor.matmul(
                            ps_o[:],
                            w2_sb[:, i, j * P : (j + 1) * P],
                            hts[i][:],
                            start=(i == 0),
                            stop=(i == HC - 1),
                        )
                    o_t = op_.tile([P, MC], f32, tag="o")
                    nc.scalar.activation(
                        o_t[:], ps_o[:], AF.Identity, bias=b2_sb[:, j : j + 1]
                    )
                    nc.vector.tensor_tensor(
                        o_t[:], o_t[:], gbc[:, m * MC : (m + 1) * MC], ALU.mult
                    )
                    nc.sync.dma_start(
                        oT_d.ap()[j * P : (j + 1) * P, m * MC : (m + 1) * MC], o_t[:]
                    )
    nc.compile()
    return nc


# ---------------- host side ----------------


def _get(name, builder):
    if name not in _CACHE:
        _CACHE[name] = builder()
    return _CACHE[name]


def _run(nc, in_maps):
    from concourse.bass_utils import run_bass_kernel_spmd

    return run_bass_kernel_spmd(nc, in_maps, core_ids=list(range(NCORE)), trace=False)


def _tile_wg_nz(w_gate, w_noise, noise):
    wgcat = np.concatenate(
        [np.asarray(w_gate, np.float32), np.asarray(w_noise, np.float32)], axis=1
    )
    wg = np.ascontiguousarray(wgcat.reshape(KC, P, 2 * E).transpose(1, 0, 2))
    nz = np.ascontiguousarray(
        np.asarray(noise, np.float32).reshape(B, E).reshape(TB, P, E).transpose(1, 0, 2)
    )
    return wg, nz


def _tile_xg(x):
    return np.ascontiguousarray(x.reshape(TB, P, KC, P).transpose(0, 3, 2, 1))


def _tile_expert(W1, b1, W2, b2, e):
    bf16 = ml_dtypes.bfloat16
    w1e = np.asarray(W1[e], np.float32).astype(bf16)
    w1t = np.ascontiguousarray(
        w1e.reshape(KC, P, HC, P).transpose(2, 1, 0, 3).reshape(HC, P, D)
    )
    w2e = np.asarray(W2[e], np.float32).astype(bf16)
    w2t = np.ascontiguousarray(w2e.reshape(HC, P, O).transpose(1, 0, 2))
    b1t = np.ascontiguousarray(np.asarray(b1[e], np.float32).reshape(HC, P).T)
    b2t = np.ascontiguousarray(np.asarray(b2[e], np.float32).reshape(OC, P).T)
    return w1t, w2t, b1t, b2t


def _finish(y, probs_full, expand_size):
    es = int(np.asarray(expand_size))
    out = np.zeros((B, es, O), np.float32)
    out[:, 0, :] = y.astype(np.float32)
    lf = probs_full.reshape(-1).astype(np.float64)
    loss = LOSS_COEF * np.var(lf, ddof=1) / (np.mean(lf) ** 2 + 1e-10)
    return out, np.float32(loss)


def _kernel_sparse(x, noise, expand_size, w_gate, w_noise, W1, b1, W2, b2):
    bf16 = ml_dtypes.bfloat16
    x = np.ascontiguousarray(np.asarray(x, dtype=np.float32))
    wg, nz = _tile_wg_nz(w_gate, w_noise, noise)
    xg = _tile_xg(x)

    # ---- phase 1: gating, token-parallel ----
    nc1 = _get("gate", _build_gate)
    in1 = []
    for i in range(NCORE):
        in1.append(
            {
                "xg": np.ascontiguousarray(xg[i * TBL : (i + 1) * TBL]),
                "wg": wg,
                "nz": np.ascontiguousarray(nz[:, i * TBL : (i + 1) * TBL, :]),
            }
        )
    r1 = _run(nc1, in1)
    gates_full = np.empty((B, E), np.float32)
    probs_full = np.empty((B, E), np.float32)
    for i in range(NCORE):
        g = r1.results[i]["gates"].reshape(P, TBL, E).transpose(1, 0, 2)
        p = r1.results[i]["probs"].reshape(P, TBL, E).transpose(1, 0, 2)
        gates_full[i * TBL * P : (i + 1) * TBL * P] = g.reshape(TBL * P, E)
        probs_full[i * TBL * P : (i + 1) * TBL * P] = p.reshape(TBL * P, E)

    # ---- host routing ----
    idxs, gvals = [], []
    for e in range(E):
        idx = np.nonzero(gates_full[:, e] > 0.0)[0]
        if len(idx) > CAP:
            return None  # overflow -> dense fallback
        idxs.append(idx)
        gvals.append(gates_full[idx, e])

    x_bf = x.astype(bf16)
    nc2 = _get("mlp", _build_mlp)
    in2 = []
    for e in range(E):
        idx = idxs[e]
        xe = np.zeros((CAP, D), bf16)
        xe[: len(idx)] = x_bf[idx]
        xbt = np.ascontiguousarray(xe.reshape(CAP, KC, P).transpose(2, 1, 0))
        gpad = np.zeros((1, CAP), np.float32)
        gpad[0, : len(idx)] = gvals[e]
        w1t, w2t, b1t, b2t = _tile_expert(W1, b1, W2, b2, e)
        in2.append(
            {
                "xbt": xbt,
                "w1": w1t,
                "w2": w2t,
                "b1": b1t,
                "b2": b2t,
                "g": gpad,
            }
        )
    r2 = _run(nc2, in2)

    y = np.zeros((B, O), np.float64)
    for e in range(E):
        cnt = len(idxs[e])
        y[idxs[e]] += r2.results[e]["oT"][:, :cnt].T.astype(np.float64)
    return _finish(y, probs_full, expand_size)


def _kernel_dense(x, noise, expand_size, w_gate, w_noise, W1, b1, W2, b2):
    bf16 = ml_dtypes.bfloat16
    x = np.ascontiguousarray(np.asarray(x, dtype=np.float32))
    wg, nz = _tile_wg_nz(w_gate, w_noise, noise)
    xg = _tile_xg(x)
    xbf = x.astype(bf16)
    xb = np.ascontiguousarray(xbf.reshape(NM, MC, KC, P).transpose(0, 3, 2, 1))

    nc = _get("dense", _build_dense)
    in_maps = []
    for e in range(E):
        w1t, w2t, b1t, b2t = _tile_expert(W1, b1, W2, b2, e)
        sel = np.zeros((1, E), np.float32)
        sel[0, e] = 1.0
        in_maps.append(
            {
                "xg": xg, "xb": xb, "w1": w1t, "w2": w2t, "b1": b1t,
                "b2": b2t, "wg": wg, "nz": nz, "sel": sel,
            }
        )
    res = _run(nc, in_maps)

    oT_sum = np.zeros((O, B), np.float64)
    for e in range(E):
        oT_sum += res.results[e]["oT"].astype(np.float64)
    y = oT_sum.T
    probs_full = (
        res.results[0]["probs"].reshape(P, TB, E).transpose(1, 0, 2).reshape(B, E)
    )
    return _finish(y, probs_full, expand_size)


def kernel(x, noise, expand_size, w_gate, w_noise, W1, b1, W2, b2):
    r = _kernel_sparse(x, noise, expand_size, w_gate, w_noise, W1, b1, W2, b2)
    if r is None:
        r = _kernel_dense(x, noise, expand_size, w_gate, w_noise, W1, b1, W2, b2)
    return r


# revision 18
# speedup vs baseline: 1.0328x; 1.0328x over previous
"""MoE (noisy top-k gating, E=8 experts, K=4) forward on 8 trn2 NeuronCores.

Sharding: expert-parallel with capacity-based token gathering.

Phase 1 (device, token-parallel): each core computes the noisy-top-k gating
for B/8 tokens (fp32 matmuls + Max8 sort + Exp/Erf activations) and returns
dense gates [B/8, E] and the top-k inclusion probabilities (for the
load-balance loss).

Host routing: from the device-computed gates, build each expert's token
list (~B*K/E tokens), pad to CAP, gather the bf16 token vectors.

Phase 2 (device, expert-parallel): core e runs the dense 2-layer MLP for
its expert over its gathered CAP tokens in bf16 (transposed-activation
dataflow xT -> hT -> oT), scales by the gathered gate row, and returns the
partial oT [O, CAP]. The host scatter-adds the 8 partials into y (the
expert-combine reduction) and finishes the scalar loss.

If any expert is assigned more than CAP tokens (never for the benchmark
shapes: observed max 2101 vs CAP 2304), a dense fallback kernel computes
all 4096 tokens on every expert.
"""

import numpy as np
import ml_dtypes

B, D, H, O, E, K = 4096, 1024, 4096, 1024, 8, 4
NOISE_EPS = 0.01
LOSS_COEF = 0.01

P = 128          # partitions
TB = B // P      # 32 token tiles of 128
KC = D // P      # 8 contraction chunks for D
HC = H // P      # 32 h tiles
OC = O // P      # 8 o tiles

NCORE = 8
TBL = TB // NCORE  # 4 token tiles per core in phase 1

CAP = 2176       # per-expert token capacity (margin over observed max 2101)
# large chunks first: chunk-0's layer-1 span covers the w2 prefetch DMA
CHUNKS = [(0, 512), (512, 512), (1024, 512), (1536, 384), (1920, 256)]

MC = 512         # dense-fallback chunk
NM = B // MC

_CACHE = {}


def _patch_act_tables():
    """Steer Exp/Ln to the combined natural_log_exp table set.

    The act-table chooser greedily picks the first set containing each
    function, bouncing exp_and_others -> natural_log -> exp_and_others ->
    sigmoid (4 x ~2.7us loads) for our Exp,Ln,Exp,Erf sequence. Hiding Exp/Ln
    from the single-function sets makes it settle on natural_log_exp (2
    loads). Only the choice changes; set ids still index act_info.json.
    """
    import concourse.bacc as bacc
    import concourse.mybir as mybir

    if getattr(bacc, "_moe_act_patched", False):
        return
    orig = bacc.get_activation_tables
    AF = mybir.ActivationFunctionType

    def patched(arch):
        t = {k: set(v) for k, v in orig(arch).items()}
        if "natural_log_exp_and_others" in t:
            t.get("exp_and_others", set()).discard(AF.Exp)
            t.get("natural_log", set()).discard(AF.Ln)
        return t

    bacc.get_activation_tables = patched
    bacc._moe_act_patched = True


def _emit_gating(nc, tc, bass, mybir, tbl, xg_d, wg_sb, nz_sb, pools):
    """Gating math for tbl token tiles. Returns (gates_all, probs_sb) sbuf APs."""
    f32 = mybir.dt.float32
    AF = mybir.ActivationFunctionType
    ALU = mybir.AluOpType
    AX = mybir.AxisListType
    pers, xgp, pg = pools

    raw_all = pers.tile([P, tbl, 2 * E], f32, tag="raw")
    sp_all = pers.tile([P, tbl, E], f32, tag="sp")
    noisy_all = pers.tile([P, tbl, E], f32, tag="noisy")
    sort_all = pers.tile([P, tbl, E], f32, tag="sort")
    diff_all = pers.tile([P, tbl, E], f32, tag="diff")
    expd_all = pers.tile([P, tbl, E], f32, tag="expd")
    mask_all = pers.tile([P, tbl, E], f32, tag="mask")
    gme_all = pers.tile([P, tbl, E], f32, tag="gme")
    gs_all = pers.tile([P, tbl], f32, tag="gs")
    rs_all = pers.tile([P, tbl], f32, tag="rs")
    zin_all = pers.tile([P, tbl, E], f32, tag="zin")
    zout_all = pers.tile([P, tbl, E], f32, tag="zout")
    rstd_all = pers.tile([P, tbl, E], f32, tag="rstd")
    min_all = pers.tile([P, tbl, E], mybir.dt.uint32, tag="min")
    pin_all = pers.tile([P, tbl, E], f32, tag="pin")
    probs_sb = pers.tile([P, tbl, E], f32, tag="probs")
    gates_all = pers.tile([P, tbl, E], f32, tag="gates")

    clean_all = raw_all[:, :, 0:E]

    for t in range(tbl):
        xg_t = xgp.tile([P, KC, P], f32, tag="xg")
        nc.sync.dma_start(xg_t[:], xg_d.ap()[t])
        ps = pg.tile([P, 2 * E], f32, tag="pg")
        for c in range(KC):
            nc.tensor.matmul(
                ps[:], xg_t[:, c, :], wg_sb[:, c, :],
                start=(c == 0), stop=(c == KC - 1),
            )
        nc.vector.tensor_copy(raw_all[:, t, :], ps[:])

    # stddev = softplus(rawnoise) + eps = ln(1 + exp(r)) + eps
    nc.scalar.activation(sp_all[:], raw_all[:, :, E : 2 * E], AF.Exp)
    nc.scalar.activation(sp_all[:], sp_all[:], AF.Ln, bias=1.0)
    nc.vector.tensor_scalar_add(sp_all[:], sp_all[:], NOISE_EPS)
    # noisy = clean + noise * stddev
    nc.vector.tensor_tensor(noisy_all[:], nz_sb[:], sp_all[:], ALU.mult)
    nc.vector.tensor_tensor(noisy_all[:], noisy_all[:], clean_all, ALU.add)
    nc.vector.reciprocal(rstd_all[:], sp_all[:])
    for t in range(tbl):
        nc.vector.max(sort_all[:, t, :], noisy_all[:, t, :])
    # batched threshold ops: broadcast the kth/(k+1)th values along E
    thr4b = sort_all[:, :, K - 1 : K].to_broadcast([P, tbl, E])
    thr5b = sort_all[:, :, K : K + 1].to_broadcast([P, tbl, E])
    nc.vector.tensor_tensor(diff_all[:], noisy_all[:], thr4b, ALU.subtract)
    nc.vector.tensor_tensor(zin_all[:], clean_all, thr5b, ALU.subtract)
    nc.vector.tensor_tensor(zout_all[:], clean_all, thr4b, ALU.subtract)
    nc.vector.tensor_tensor(min_all[:], noisy_all[:], thr5b, ALU.is_gt)
    nc.scalar.activation(expd_all[:], diff_all[:], AF.Exp)
    nc.vector.tensor_scalar(mask_all[:], diff_all[:], 0.0, None, op0=ALU.is_ge)
    nc.vector.tensor_tensor(gme_all[:], expd_all[:], mask_all[:], ALU.mult)
    nc.vector.tensor_reduce(gs_all[:], gme_all[:], axis=AX.X, op=ALU.add)
    nc.vector.reciprocal(rs_all[:], gs_all[:])
    rsb = rs_all[:, :, None].to_broadcast([P, tbl, E])
    nc.vector.tensor_tensor(gates_all[:], gme_all[:], rsb, ALU.mult)
    # prob = Phi(z) = 0.5 * erf(z / sqrt(2)) + 0.5
    nc.vector.tensor_tensor(zin_all[:], zin_all[:], rstd_all[:], ALU.mult)
    nc.vector.tensor_tensor(zout_all[:], zout_all[:], rstd_all[:], ALU.mult)
    isq2 = float(1.0 / np.sqrt(2.0))
    nc.scalar.activation(pin_all[:], zin_all[:], AF.Erf, scale=isq2)
    nc.scalar.activation(probs_sb[:], zout_all[:], AF.Erf, scale=isq2)
    nc.vector.tensor_scalar(
        pin_all[:], pin_all[:], 0.5, 0.5, op0=ALU.mult, op1=ALU.add
    )
    nc.vector.tensor_scalar(
        probs_sb[:], probs_sb[:], 0.5, 0.5, op0=ALU.mult, op1=ALU.add
    )
    nc.vector.copy_predicated(probs_sb[:], min_all[:], pin_all[:])
    return gates_all, probs_sb


def _build_gate():
    """Phase-1: token-parallel gating; each core handles B/8 tokens."""
    import concourse.bacc as bacc
    import concourse.bass as bass
    import concourse.mybir as mybir
    import concourse.tile as tile

    _patch_act_tables()
    f32 = mybir.dt.float32
    nc = bacc.Bacc("TRN2", target_bir_lowering=False, debug=False)

    xg_d = nc.dram_tensor("xg", [TBL, P, KC, P], f32, kind="ExternalInput")
    wg_d = nc.dram_tensor("wg", [P, KC, 2 * E], f32, kind="ExternalInput")
    nz_d = nc.dram_tensor("nz", [P, TBL, E], f32, kind="ExternalInput")
    gates_d = nc.dram_tensor("gates", [P, TBL * E], f32, kind="ExternalOutput")
    probs_d = nc.dram_tensor("probs", [P, TBL * E], f32, kind="ExternalOutput")

    with tile.TileContext(nc) as tc:
        with (
            tc.tile_pool(name="persist", bufs=1) as pers,
            tc.tile_pool(name="xgp", bufs=3) as xgp,
            tc.tile_pool(name="pg", bufs=2, space="PSUM") as pg,
        ):
            wg_sb = pers.tile([P, KC, 2 * E], f32, tag="wg")
            nc.sync.dma_start(wg_sb[:], wg_d.ap())
            nz_sb = pers.tile([P, TBL, E], f32, tag="nz")
            nc.sync.dma_start(nz_sb[:], nz_d.ap())
            gates_all, probs_sb = _emit_gating(
                nc, tc, bass, mybir, TBL, xg_d, wg_sb, nz_sb, (pers, xgp, pg)
            )
            nc.sync.dma_start(gates_d.ap(), gates_all.rearrange("p t e -> p (t e)"))
            nc.sync.dma_start(probs_d.ap(), probs_sb.rearrange("p t e -> p (t e)"))
    nc.compile()
    return nc


def _build_mlp():
    """Phase-2: expert-parallel MLP over CAP gathered tokens per core."""
    import concourse.bacc as bacc
    import concourse.bass as bass
    import concourse.mybir as mybir
    import concourse.tile as tile

    f32 = mybir.dt.float32
    bf16 = mybir.dt.bfloat16
    AF = mybir.ActivationFunctionType
    ALU = mybir.AluOpType

    _patch_act_tables()
    nc = bacc.Bacc("TRN2", target_bir_lowering=False, debug=False)

    xbt_d = nc.dram_tensor("xbt", [P, KC, CAP], bf16, kind="ExternalInput")
    w1_d = nc.dram_tensor("w1", [HC, P, D], bf16, kind="ExternalInput")
    w2_d = nc.dram_tensor("w2", [P, HC, O], bf16, kind="ExternalInput")
    b1_d = nc.dram_tensor("b1", [P, HC], f32, kind="ExternalInput")
    b2_d = nc.dram_tensor("b2", [P, OC], f32, kind="ExternalInput")
    g_d = nc.dram_tensor("g", [1, CAP], f32, kind="ExternalInput")
    oT_d = nc.dram_tensor("oT", [O, CAP], f32, kind="ExternalOutput")

    with tile.TileContext(nc) as tc:
        with (
            tc.tile_pool(name="persist", bufs=1) as pers,
            tc.tile_pool(name="xbp", bufs=2) as xbp,
            tc.tile_pool(name="w1p", bufs=4) as w1p,
            tc.tile_pool(name="hp", bufs=36) as hp,
            tc.tile_pool(name="op", bufs=4) as op_,
            tc.tile_pool(name="ph", bufs=3, space="PSUM") as ph,
            tc.tile_pool(name="po", bufs=3, space="PSUM") as po,
        ):
            b1_sb = pers.tile([P, HC], f32, tag="b1")
            nc.sync.dma_start(b1_sb[:], b1_d.ap())
            b2_sb = pers.tile([P, OC], f32, tag="b2")
            nc.sync.dma_start(b2_sb[:], b2_d.ap())
            gbc = pers.tile([P, CAP], f32, tag="gbc")
            nc.gpsimd.dma_start(
                out=gbc[:], in_=bass.AP(tensor=g_d, offset=0, ap=[[0, P], [1, CAP]])
            )
            # w2 is DMA'd in quarters, emitted after the first chunk's layer-1
            # so the startup DMAs that gate the first matmuls go first.
            w2_sb = pers.tile([P, HC, O], bf16, tag="w2")
            w2_started = False

            for ci, (start, sz) in enumerate(CHUNKS):
                xb_t = xbp.tile([P, KC, 512], bf16, tag="xb", name="xb_t")[:, :, :sz]
                # issue order drives the DMA queue: the first matmul's inputs
                # (w1 tile 0 first half, xb k-slice 0) must be issued first
                w1_0 = w1p.tile([P, D], bf16, tag="w1", name="w1_0")
                nc.sync.dma_start(w1_0[:, : D // 2], w1_d.ap()[0][:, : D // 2])
                nc.sync.dma_start(xb_t[:, 0, :], xbt_d.ap()[:, 0, start : start + sz])
                nc.sync.dma_start(w1_0[:, D // 2 :], w1_d.ap()[0][:, D // 2 :])
                for c in range(1, KC):
                    nc.sync.dma_start(
                        xb_t[:, c, :], xbt_d.ap()[:, c, start : start + sz]
                    )
                hts = []
                for i in range(HC):
                    if i == 0:
                        w1_t = w1_0
                    else:
                        w1_t = w1p.tile([P, D], bf16, tag="w1")
                        nc.sync.dma_start(
                            w1_t[:, : D // 2], w1_d.ap()[i][:, : D // 2]
                        )
                        nc.sync.dma_start(
                            w1_t[:, D // 2 :], w1_d.ap()[i][:, D // 2 :]
                        )
                    ps_h = ph.tile([P, 512], f32, tag="ph", name="ps_h")[:, :sz]
                    for c in range(KC):
                        nc.tensor.matmul(
                            ps_h[:],
                            w1_t[:, c * P : (c + 1) * P],
                            xb_t[:, c, :],
                            start=(c == 0),
                            stop=(c == KC - 1),
                        )
                    h_t = hp.tile([P, 512], bf16, tag="h", name="h_t")[:, :sz]
                    nc.scalar.activation(
                        h_t[:], ps_h[:], AF.Relu, bias=b1_sb[:, i : i + 1]
                    )
                    hts.append(h_t)
                if not w2_started:
                    w2_started = True
                    q = HC // 4
                    for qi in range(4):
                        nc.sync.dma_start(
                            w2_sb[:, qi * q : (qi + 1) * q, :],
                            w2_d.ap()[:, qi * q : (qi + 1) * q, :],
                        )
                for j in range(OC):
                    ps_o = po.tile([P, 512], f32, tag="po", name="ps_o")[:, :sz]
                    for i in range(HC):
                        nc.tensor.matmul(
                            ps_o[:],
                            w2_sb[:, i, j * P : (j + 1) * P],
                            hts[i][:],
                            start=(i == 0),
                            stop=(i == HC - 1),
                        )
                    o_t = op_.tile([P, 512], f32, tag="o", name="o_t")[:, :sz]
                    nc.scalar.activation(
                        o_t[:], ps_o[:], AF.Identity, bias=b2_sb[:, j : j + 1]
                    )
                    nc.vector.tensor_tensor(
                        o_t[:], o_t[:], gbc[:, start : start + sz], ALU.mult
                    )
                    nc.sync.dma_start(
                        oT_d.ap()[j * P : (j + 1) * P, start : start + sz], o_t[:]
                    )
    nc.compile()
    return nc


def _build_dense():
    """Fallback: every core computes its expert densely on all B tokens."""
    import concourse.bacc as bacc
    import concourse.bass as bass
    import concourse.mybir as mybir
    import concourse.tile as tile
    from concourse.masks import make_identity

    f32 = mybir.dt.float32
    bf16 = mybir.dt.bfloat16
    AF = mybir.ActivationFunctionType
    ALU = mybir.AluOpType
    AX = mybir.AxisListType

    _patch_act_tables()
    nc = bacc.Bacc("TRN2", target_bir_lowering=False, debug=False)

    xg_d = nc.dram_tensor("xg", [TB, P, KC, P], f32, kind="ExternalInput")
    xb_d = nc.dram_tensor("xb", [NM, P, KC, MC], bf16, kind="ExternalInput")
    w1_d = nc.dram_tensor("w1", [HC, P, D], bf16, kind="ExternalInput")
    w2_d = nc.dram_tensor("w2", [P, HC, O], bf16, kind="ExternalInput")
    b1_d = nc.dram_tensor("b1", [P, HC], f32, kind="ExternalInput")
    b2_d = nc.dram_tensor("b2", [P, OC], f32, kind="ExternalInput")
    wg_d = nc.dram_tensor("wg", [P, KC, 2 * E], f32, kind="ExternalInput")
    nz_d = nc.dram_tensor("nz", [P, TB, E], f32, kind="ExternalInput")
    sel_d = nc.dram_tensor("sel", [1, E], f32, kind="ExternalInput")
    oT_d = nc.dram_tensor("oT", [O, B], f32, kind="ExternalOutput")
    probs_d = nc.dram_tensor("probs", [P, TB * E], f32, kind="ExternalOutput")
    grow_d = nc.dram_tensor("grow", [TB, P], f32)

    with tile.TileContext(nc) as tc:
        with (
            tc.tile_pool(name="persist", bufs=1) as pers,
            tc.tile_pool(name="xgp", bufs=3) as xgp,
            tc.tile_pool(name="xbp", bufs=2) as xbp,
            tc.tile_pool(name="w1p", bufs=4) as w1p,
            tc.tile_pool(name="hp", bufs=36) as hp,
            tc.tile_pool(name="op", bufs=4) as op_,
            tc.tile_pool(name="pg", bufs=2, space="PSUM") as pg,
            tc.tile_pool(name="ptr", bufs=1, space="PSUM") as ptr,
            tc.tile_pool(name="ph", bufs=2, space="PSUM") as ph,
            tc.tile_pool(name="po", bufs=2, space="PSUM") as po,
        ):
            b1_sb = pers.tile([P, HC], f32, tag="b1")
            nc.sync.dma_start(b1_sb[:], b1_d.ap())
            b2_sb = pers.tile([P, OC], f32, tag="b2")
            nc.sync.dma_start(b2_sb[:], b2_d.ap())
            wg_sb = pers.tile([P, KC, 2 * E], f32, tag="wg")
            nc.sync.dma_start(wg_sb[:], wg_d.ap())
            nz_sb = pers.tile([P, TB, E], f32, tag="nz")
            nc.sync.dma_start(nz_sb[:], nz_d.ap())
            sel_sb = pers.tile([P, E], f32, tag="sel")
            nc.gpsimd.dma_start(
                out=sel_sb[:],
                in_=bass.AP(tensor=sel_d, offset=0, ap=[[0, P], [1, E]]),
            )
            ident = pers.tile([P, P], f32, tag="ident")
            make_identity(nc, ident)
            w2_sb = pers.tile([P, HC, O], bf16, tag="w2")
            nc.sync.dma_start(w2_sb[:], w2_d.ap())
            gcol = pers.tile([P, TB], f32, tag="gcol")
            gT_sb = pers.tile([TB, P], f32, tag="gT")
            gbc = pers.tile([P, B], f32, tag="gbc")

            gates_all, probs_sb = _emit_gating(
                nc, tc, bass, mybir, TB, xg_d, wg_sb, nz_sb, (pers, xgp, pg)
            )
            nc.sync.dma_start(probs_d.ap(), probs_sb.rearrange("p t e -> p (t e)"))
            # own-expert gate column: dot(gates, sel)
            gsel = pers.tile([P, TB, E], f32, tag="gsel")
            selb = sel_sb[:, None, :].to_broadcast([P, TB, E])
            nc.vector.tensor_tensor(gsel[:], gates_all[:], selb, ALU.mult)
            nc.vector.tensor_reduce(gcol[:], gsel[:], axis=AX.X, op=ALU.add)
            # broadcast gcol across partitions via transpose + DRAM round-trip
            ps_tr = ptr.tile([TB, P], f32, tag="ptr")
            nc.tensor.transpose(ps_tr[:], gcol[:], ident[:])
            nc.vector.tensor_copy(gT_sb[:], ps_tr[:])
            nc.sync.dma_start(grow_d.ap(), gT_sb[:])
            nc.gpsimd.dma_start(
                out=gbc[:], in_=bass.AP(tensor=grow_d, offset=0, ap=[[0, P], [1, B]])
            )

            for m in range(NM):
                xb_t = xbp.tile([P, KC, MC], bf16, tag="xb")
                nc.sync.dma_start(xb_t[:], xb_d.ap()[m])
                hts = []
                for i in range(HC):
                    w1_t = w1p.tile([P, D], bf16, tag="w1")
                    nc.sync.dma_start(w1_t[:], w1_d.ap()[i])
                    ps_h = ph.tile([P, MC], f32, tag="ph")
                    for c in range(KC):
                        nc.tensor.matmul(
                            ps_h[:],
                            w1_t[:, c * P : (c + 1) * P],
                            xb_t[:, c, :],
                            start=(c == 0),
                            stop=(c == KC - 1),
                        )
                    h_t = hp.tile([P, MC], bf16, tag="h")
                    nc.scalar.activation(
                        h_t[:], ps_h[:], AF.Relu, bias=b1_sb[:, i : i + 1]
                    )
                    hts.append(h_t)
                for j in range(OC):
                    ps_o = po.tile([P, MC], f32, tag="po")
                    for i in range(HC):
                        nc.tensor.matmul(
                            ps_o[:],
                            w2_sb[:, i, j * P : (j + 1) * P],
                            hts[i][:],
                            start=(i == 0),
                            stop=(i == HC - 1),
                        )
                    o_t = op_.tile([P, MC], f32, tag="o")
                    nc.scalar.activation(
                        o_t[:], ps_o[:], AF.Identity, bias=b2_sb[:, j : j + 1]
                    )
                    nc.vector.tensor_tensor(
                        o_t[:], o_t[:], gbc[:, m * MC : (m + 1) * MC], ALU.mult
                    )
                    nc.sync.dma_start(
                        oT_d.ap()[j * P : (j + 1) * P, m * MC : (m + 1) * MC], o_t[:]
                    )
    nc.compile()
    return nc


# ---------------- host side ----------------


def _get(name, builder):
    if name not in _CACHE:
        _CACHE[name] = builder()
    return _CACHE[name]


def _run(nc, in_maps):
    from concourse.bass_utils import run_bass_kernel_spmd

    return run_bass_kernel_spmd(nc, in_maps, core_ids=list(range(NCORE)), trace=False)


def _tile_wg_nz(w_gate, w_noise, noise):
    wgcat = np.concatenate(
        [np.asarray(w_gate, np.float32), np.asarray(w_noise, np.float32)], axis=1
    )
    wg = np.ascontiguousarray(wgcat.reshape(KC, P, 2 * E).transpose(1, 0, 2))
    nz = np.ascontiguousarray(
        np.asarray(noise, np.float32).reshape(B, E).reshape(TB, P, E).transpose(1, 0, 2)
    )
    return wg, nz


def _tile_xg(x):
    return np.ascontiguousarray(x.reshape(TB, P, KC, P).transpose(0, 3, 2, 1))


def _tile_expert(W1, b1, W2, b2, e):
    bf16 = ml_dtypes.bfloat16
    w1e = np.asarray(W1[e], np.float32).astype(bf16)
    w1t = np.ascontiguousarray(
        w1e.reshape(KC, P, HC, P).transpose(2, 1, 0, 3).reshape(HC, P, D)
    )
    w2e = np.asarray(W2[e], np.float32).astype(bf16)
    w2t = np.ascontiguousarray(w2e.reshape(HC, P, O).transpose(1, 0, 2))
    b1t = np.ascontiguousarray(np.asarray(b1[e], np.float32).reshape(HC, P).T)
    b2t = np.ascontiguousarray(np.asarray(b2[e], np.float32).reshape(OC, P).T)
    return w1t, w2t, b1t, b2t


def _finish(y, probs_full, expand_size):
    es = int(np.asarray(expand_size))
    out = np.zeros((B, es, O), np.float32)
    out[:, 0, :] = y.astype(np.float32)
    lf = probs_full.reshape(-1).astype(np.float64)
    loss = LOSS_COEF * np.var(lf, ddof=1) / (np.mean(lf) ** 2 + 1e-10)
    return out, np.float32(loss)


def _kernel_sparse(x, noise, expand_size, w_gate, w_noise, W1, b1, W2, b2):
    bf16 = ml_dtypes.bfloat16
    x = np.ascontiguousarray(np.asarray(x, dtype=np.float32))
    wg, nz = _tile_wg_nz(w_gate, w_noise, noise)
    xg = _tile_xg(x)

    # ---- phase 1: gating, token-parallel ----
    nc1 = _get("gate", _build_gate)
    in1 = []
    for i in range(NCORE):
        in1.append(
            {
                "xg": np.ascontiguousarray(xg[i * TBL : (i + 1) * TBL]),
                "wg": wg,
                "nz": np.ascontiguousarray(nz[:, i * TBL : (i + 1) * TBL, :]),
            }
        )
    r1 = _run(nc1, in1)
    gates_full = np.empty((B, E), np.float32)
    probs_full = np.empty((B, E), np.float32)
    for i in range(NCORE):
        g = r1.results[i]["gates"].reshape(P, TBL, E).transpose(1, 0, 2)
        p = r1.results[i]["probs"].reshape(P, TBL, E).transpose(1, 0, 2)
        gates_full[i * TBL * P : (i + 1) * TBL * P] = g.reshape(TBL * P, E)
        probs_full[i * TBL * P : (i + 1) * TBL * P] = p.reshape(TBL * P, E)

    # ---- host routing ----
    idxs, gvals = [], []
    for e in range(E):
        idx = np.nonzero(gates_full[:, e] > 0.0)[0]
        if len(idx) > CAP:
            return None  # overflow -> dense fallback
        idxs.append(idx)
        gvals.append(gates_full[idx, e])

    x_bf = x.astype(bf16)
    nc2 = _get("mlp", _build_mlp)
    in2 = []
    for e in range(E):
        idx = idxs[e]
        xe = np.zeros((CAP, D), bf16)
        xe[: len(idx)] = x_bf[idx]
        xbt = np.ascontiguousarray(xe.reshape(CAP, KC, P).transpose(2, 1, 0))
        gpad = np.zeros((1, CAP), np.float32)
        gpad[0, : len(idx)] = gvals[e]
        w1t, w2t, b1t, b2t = _tile_expert(W1, b1, W2, b2, e)
        in2.append(
            {
                "xbt": xbt,
                "w1": w1t,
                "w2": w2t,
                "b1": b1t,
                "b2": b2t,
                "g": gpad,
            }
        )
    r2 = _run(nc2, in2)

    y = np.zeros((B, O), np.float64)
    for e in range(E):
        cnt = len(idxs[e])
        y[idxs[e]] += r2.results[e]["oT"][:, :cnt].T.astype(np.float64)
    return _finish(y, probs_full, expand_size)


def _kernel_dense(x, noise, expand_size, w_gate, w_noise, W1, b1, W2, b2):
    bf16 = ml_dtypes.bfloat16
    x = np.ascontiguousarray(np.asarray(x, dtype=np.float32))
    wg, nz = _tile_wg_nz(w_gate, w_noise, noise)
    xg = _tile_xg(x)
    xbf = x.astype(bf16)
    xb = np.ascontiguousarray(xbf.reshape(NM, MC, KC, P).transpose(0, 3, 2, 1))

    nc = _get("dense", _build_dense)
    in_maps = []
    for e in range(E):
        w1t, w2t, b1t, b2t = _tile_expert(W1, b1, W2, b2, e)
        sel = np.zeros((1, E), np.float32)
        sel[0, e] = 1.0
        in_maps.append(
            {
                "xg": xg, "xb": xb, "w1": w1t, "w2": w2t, "b1": b1t,
                "b2": b2t, "wg": wg, "nz": nz, "sel": sel,
            }
        )
    res = _run(nc, in_maps)

    oT_sum = np.zeros((O, B), np.float64)
    for e in range(E):
        oT_sum += res.results[e]["oT"].astype(np.float64)
    y = oT_sum.T
    probs_full = (
        res.results[0]["probs"].reshape(P, TB, E).transpose(1, 0, 2).reshape(B, E)
    )
    return _finish(y, probs_full, expand_size)


def kernel(x, noise, expand_size, w_gate, w_noise, W1, b1, W2, b2):
    r = _kernel_sparse(x, noise, expand_size, w_gate, w_noise, W1, b1, W2, b2)
    if r is None:
        r = _kernel_dense(x, noise, expand_size, w_gate, w_noise, W1, b1, W2, b2)
    return r


# revision 19
# speedup vs baseline: 1.0405x; 1.0074x over previous
"""MoE (noisy top-k gating, E=8 experts, K=4) forward on 8 trn2 NeuronCores.

Sharding: expert-parallel with capacity-based token gathering.

Phase 1 (device, token-parallel): each core computes the noisy-top-k gating
for B/8 tokens (fp32 matmuls + Max8 sort + Exp/Erf activations) and returns
dense gates [B/8, E] and the top-k inclusion probabilities (for the
load-balance loss).

Host routing: from the device-computed gates, build each expert's token
list (~B*K/E tokens), pad to CAP, gather the bf16 token vectors.

Phase 2 (device, expert-parallel): core e runs the dense 2-layer MLP for
its expert over its gathered CAP tokens in bf16 (transposed-activation
dataflow xT -> hT -> oT), scales by the gathered gate row, and returns the
partial oT [O, CAP]. The host scatter-adds the 8 partials into y (the
expert-combine reduction) and finishes the scalar loss.

If any expert is assigned more than CAP tokens (never for the benchmark
shapes: observed max 2101 vs CAP 2304), a dense fallback kernel computes
all 4096 tokens on every expert.
"""

import numpy as np
import ml_dtypes

B, D, H, O, E, K = 4096, 1024, 4096, 1024, 8, 4
NOISE_EPS = 0.01
LOSS_COEF = 0.01

P = 128          # partitions
TB = B // P      # 32 token tiles of 128
KC = D // P      # 8 contraction chunks for D
HC = H // P      # 32 h tiles
OC = O // P      # 8 o tiles

NCORE = 8
TBL = TB // NCORE  # 4 token tiles per core in phase 1

CAP = 2176       # per-expert token capacity (margin over observed max 2101)
# large chunks first: chunk-0's layer-1 span covers the w2 prefetch DMA
CHUNKS = [(0, 512), (512, 512), (1024, 512), (1536, 384), (1920, 256)]

MC = 512         # dense-fallback chunk
NM = B // MC

_CACHE = {}


def _patch_act_tables():
    """Steer Exp/Ln to the combined natural_log_exp table set.

    The act-table chooser greedily picks the first set containing each
    function, bouncing exp_and_others -> natural_log -> exp_and_others ->
    sigmoid (4 x ~2.7us loads) for our Exp,Ln,Exp,Erf sequence. Hiding Exp/Ln
    from the single-function sets makes it settle on natural_log_exp (2
    loads). Only the choice changes; set ids still index act_info.json.
    """
    import concourse.bacc as bacc
    import concourse.mybir as mybir

    if getattr(bacc, "_moe_act_patched", False):
        return
    orig = bacc.get_activation_tables
    AF = mybir.ActivationFunctionType

    def patched(arch):
        t = {k: set(v) for k, v in orig(arch).items()}
        if "natural_log_exp_and_others" in t:
            t.get("exp_and_others", set()).discard(AF.Exp)
            t.get("natural_log", set()).discard(AF.Ln)
        return t

    bacc.get_activation_tables = patched
    bacc._moe_act_patched = True


def _emit_gating(nc, tc, bass, mybir, tbl, xg_d, wg_sb, nz_sb, pools):
    """Gating math for tbl token tiles. Returns (gates_all, probs_sb) sbuf APs."""
    f32 = mybir.dt.float32
    AF = mybir.ActivationFunctionType
    ALU = mybir.AluOpType
    AX = mybir.AxisListType
    pers, xgp, pg = pools

    raw_all = pers.tile([P, tbl, 2 * E], f32, tag="raw")
    sp_all = pers.tile([P, tbl, E], f32, tag="sp")
    noisy_all = pers.tile([P, tbl, E], f32, tag="noisy")
    sort_all = pers.tile([P, tbl, E], f32, tag="sort")
    diff_all = pers.tile([P, tbl, E], f32, tag="diff")
    expd_all = pers.tile([P, tbl, E], f32, tag="expd")
    mask_all = pers.tile([P, tbl, E], f32, tag="mask")
    gme_all = pers.tile([P, tbl, E], f32, tag="gme")
    gs_all = pers.tile([P, tbl], f32, tag="gs")
    rs_all = pers.tile([P, tbl], f32, tag="rs")
    zin_all = pers.tile([P, tbl, E], f32, tag="zin")
    zout_all = pers.tile([P, tbl, E], f32, tag="zout")
    rstd_all = pers.tile([P, tbl, E], f32, tag="rstd")
    min_all = pers.tile([P, tbl, E], mybir.dt.uint32, tag="min")
    pin_all = pers.tile([P, tbl, E], f32, tag="pin")
    probs_sb = pers.tile([P, tbl, E], f32, tag="probs")
    gates_all = pers.tile([P, tbl, E], f32, tag="gates")

    clean_all = raw_all[:, :, 0:E]

    for t in range(tbl):
        xg_t = xgp.tile([P, KC, P], f32, tag="xg")
        nc.sync.dma_start(xg_t[:], xg_d.ap()[t])
        ps = pg.tile([P, 2 * E], f32, tag="pg")
        for c in range(KC):
            nc.tensor.matmul(
                ps[:], xg_t[:, c, :], wg_sb[:, c, :],
                start=(c == 0), stop=(c == KC - 1),
            )
        nc.vector.tensor_copy(raw_all[:, t, :], ps[:])

    # stddev = softplus(rawnoise) + eps = ln(1 + exp(r)) + eps
    nc.scalar.activation(sp_all[:], raw_all[:, :, E : 2 * E], AF.Exp)
    nc.scalar.activation(sp_all[:], sp_all[:], AF.Ln, bias=1.0)
    nc.vector.tensor_scalar_add(sp_all[:], sp_all[:], NOISE_EPS)
    # noisy = clean + noise * stddev
    nc.vector.tensor_tensor(noisy_all[:], nz_sb[:], sp_all[:], ALU.mult)
    nc.vector.tensor_tensor(noisy_all[:], noisy_all[:], clean_all, ALU.add)
    nc.vector.reciprocal(rstd_all[:], sp_all[:])
    for t in range(tbl):
        nc.vector.max(sort_all[:, t, :], noisy_all[:, t, :])
    # batched threshold ops: broadcast the kth/(k+1)th values along E
    thr4b = sort_all[:, :, K - 1 : K].to_broadcast([P, tbl, E])
    thr5b = sort_all[:, :, K : K + 1].to_broadcast([P, tbl, E])
    nc.vector.tensor_tensor(diff_all[:], noisy_all[:], thr4b, ALU.subtract)
    nc.vector.tensor_tensor(zin_all[:], clean_all, thr5b, ALU.subtract)
    nc.vector.tensor_tensor(zout_all[:], clean_all, thr4b, ALU.subtract)
    nc.vector.tensor_tensor(min_all[:], noisy_all[:], thr5b, ALU.is_gt)
    nc.scalar.activation(expd_all[:], diff_all[:], AF.Exp)
    nc.vector.tensor_scalar(mask_all[:], diff_all[:], 0.0, None, op0=ALU.is_ge)
    nc.vector.tensor_tensor(gme_all[:], expd_all[:], mask_all[:], ALU.mult)
    nc.vector.tensor_reduce(gs_all[:], gme_all[:], axis=AX.X, op=ALU.add)
    nc.vector.reciprocal(rs_all[:], gs_all[:])
    rsb = rs_all[:, :, None].to_broadcast([P, tbl, E])
    nc.vector.tensor_tensor(gates_all[:], gme_all[:], rsb, ALU.mult)
    # prob = Phi(z) = 0.5 * erf(z / sqrt(2)) + 0.5
    nc.vector.tensor_tensor(zin_all[:], zin_all[:], rstd_all[:], ALU.mult)
    nc.vector.tensor_tensor(zout_all[:], zout_all[:], rstd_all[:], ALU.mult)
    isq2 = float(1.0 / np.sqrt(2.0))
    nc.scalar.activation(pin_all[:], zin_all[:], AF.Erf, scale=isq2)
    nc.scalar.activation(probs_sb[:], zout_all[:], AF.Erf, scale=isq2)
    nc.vector.tensor_scalar(
        pin_all[:], pin_all[:], 0.5, 0.5, op0=ALU.mult, op1=ALU.add
    )
    nc.vector.tensor_scalar(
        probs_sb[:], probs_sb[:], 0.5, 0.5, op0=ALU.mult, op1=ALU.add
    )
    nc.vector.copy_predicated(probs_sb[:], min_all[:], pin_all[:])
    return gates_all, probs_sb


def _build_gate():
    """Phase-1: token-parallel gating; each core handles B/8 tokens."""
    import concourse.bacc as bacc
    import concourse.bass as bass
    import concourse.mybir as mybir
    import concourse.tile as tile

    _patch_act_tables()
    f32 = mybir.dt.float32
    nc = bacc.Bacc("TRN2", target_bir_lowering=False, debug=False)

    xg_d = nc.dram_tensor("xg", [TBL, P, KC, P], f32, kind="ExternalInput")
    wg_d = nc.dram_tensor("wg", [P, KC, 2 * E], f32, kind="ExternalInput")
    nz_d = nc.dram_tensor("nz", [P, TBL, E], f32, kind="ExternalInput")
    gates_d = nc.dram_tensor("gates", [P, TBL * E], f32, kind="ExternalOutput")
    probs_d = nc.dram_tensor("probs", [P, TBL * E], f32, kind="ExternalOutput")

    with tile.TileContext(nc) as tc:
        with (
            tc.tile_pool(name="persist", bufs=1) as pers,
            tc.tile_pool(name="xgp", bufs=3) as xgp,
            tc.tile_pool(name="pg", bufs=2, space="PSUM") as pg,
        ):
            wg_sb = pers.tile([P, KC, 2 * E], f32, tag="wg")
            nc.sync.dma_start(wg_sb[:], wg_d.ap())
            nz_sb = pers.tile([P, TBL, E], f32, tag="nz")
            nc.sync.dma_start(nz_sb[:], nz_d.ap())
            gates_all, probs_sb = _emit_gating(
                nc, tc, bass, mybir, TBL, xg_d, wg_sb, nz_sb, (pers, xgp, pg)
            )
            nc.sync.dma_start(gates_d.ap(), gates_all.rearrange("p t e -> p (t e)"))
            nc.sync.dma_start(probs_d.ap(), probs_sb.rearrange("p t e -> p (t e)"))
    nc.compile()
    return nc


def _build_mlp():
    """Phase-2: expert-parallel MLP over CAP gathered tokens per core."""
    import concourse.bacc as bacc
    import concourse.bass as bass
    import concourse.mybir as mybir
    import concourse.tile as tile

    f32 = mybir.dt.float32
    bf16 = mybir.dt.bfloat16
    AF = mybir.ActivationFunctionType
    ALU = mybir.AluOpType

    _patch_act_tables()
    nc = bacc.Bacc("TRN2", target_bir_lowering=False, debug=False)

    xbt_d = nc.dram_tensor("xbt", [P, KC, CAP], bf16, kind="ExternalInput")
    w1_d = nc.dram_tensor("w1", [HC, P, D], bf16, kind="ExternalInput")
    w2_d = nc.dram_tensor("w2", [P, HC, O], bf16, kind="ExternalInput")
    b1_d = nc.dram_tensor("b1", [P, HC], f32, kind="ExternalInput")
    b2_d = nc.dram_tensor("b2", [P, OC], f32, kind="ExternalInput")
    g_d = nc.dram_tensor("g", [1, CAP], f32, kind="ExternalInput")
    oT_d = nc.dram_tensor("oT", [O, CAP], f32, kind="ExternalOutput")

    with tile.TileContext(nc) as tc:
        with (
            tc.tile_pool(name="persist", bufs=1) as pers,
            tc.tile_pool(name="xbp", bufs=2) as xbp,
            tc.tile_pool(name="w1p", bufs=4) as w1p,
            tc.tile_pool(name="hp", bufs=36) as hp,
            tc.tile_pool(name="op", bufs=4) as op_,
            tc.tile_pool(name="ph", bufs=3, space="PSUM") as ph,
            tc.tile_pool(name="po", bufs=3, space="PSUM") as po,
        ):
            b1_sb = pers.tile([P, HC], f32, tag="b1")
            nc.sync.dma_start(b1_sb[:], b1_d.ap())
            b2_sb = pers.tile([P, OC], f32, tag="b2")
            nc.sync.dma_start(b2_sb[:], b2_d.ap())
            gbc = pers.tile([P, CAP], f32, tag="gbc")
            nc.gpsimd.dma_start(
                out=gbc[:], in_=bass.AP(tensor=g_d, offset=0, ap=[[0, P], [1, CAP]])
            )
            # w2 is DMA'd in quarters, emitted after the first chunk's layer-1
            # so the startup DMAs that gate the first matmuls go first.
            w2_sb = pers.tile([P, HC, O], bf16, tag="w2")
            w2_started = False
            # first HC/2 w1 tiles stay resident: halves the per-chunk w1
            # streaming rate, which the narrow chunks cannot otherwise sustain
            HR = HC // 2
            w1r = pers.tile([P, HR, D], bf16, tag="w1r")

            for ci, (start, sz) in enumerate(CHUNKS):
                xb_t = xbp.tile([P, KC, 512], bf16, tag="xb", name="xb_t")[:, :, :sz]
                # issue order drives the DMA queue: the first matmul's inputs
                # (w1 tile 0 first half, xb k-slice 0) must be issued first
                if ci == 0:
                    nc.sync.dma_start(w1r[:, 0, : D // 2], w1_d.ap()[0][:, : D // 2])
                nc.sync.dma_start(xb_t[:, 0, :], xbt_d.ap()[:, 0, start : start + sz])
                if ci == 0:
                    nc.sync.dma_start(w1r[:, 0, D // 2 :], w1_d.ap()[0][:, D // 2 :])
                for c in range(1, KC):
                    nc.sync.dma_start(
                        xb_t[:, c, :], xbt_d.ap()[:, c, start : start + sz]
                    )
                hts = []
                for i in range(HC):
                    if i < HR:
                        w1_t = w1r[:, i, :]
                        if ci == 0 and i > 0:
                            nc.sync.dma_start(
                                w1_t[:, : D // 2], w1_d.ap()[i][:, : D // 2]
                            )
                            nc.sync.dma_start(
                                w1_t[:, D // 2 :], w1_d.ap()[i][:, D // 2 :]
                            )
                    else:
                        w1_t = w1p.tile([P, D], bf16, tag="w1")
                        nc.sync.dma_start(
                            w1_t[:, : D // 2], w1_d.ap()[i][:, : D // 2]
                        )
                        nc.sync.dma_start(
                            w1_t[:, D // 2 :], w1_d.ap()[i][:, D // 2 :]
                        )
                    ps_h = ph.tile([P, 512], f32, tag="ph", name="ps_h")[:, :sz]
                    for c in range(KC):
                        nc.tensor.matmul(
                            ps_h[:],
                            w1_t[:, c * P : (c + 1) * P],
                            xb_t[:, c, :],
                            start=(c == 0),
                            stop=(c == KC - 1),
                        )
                    h_t = hp.tile([P, 512], bf16, tag="h", name="h_t")[:, :sz]
                    nc.scalar.activation(
                        h_t[:], ps_h[:], AF.Relu, bias=b1_sb[:, i : i + 1]
                    )
                    hts.append(h_t)
                if not w2_started:
                    w2_started = True
                    q = HC // 4
                    for qi in range(4):
                        nc.sync.dma_start(
                            w2_sb[:, qi * q : (qi + 1) * q, :],
                            w2_d.ap()[:, qi * q : (qi + 1) * q, :],
                        )
                for j in range(OC):
                    ps_o = po.tile([P, 512], f32, tag="po", name="ps_o")[:, :sz]
                    for i in range(HC):
                        nc.tensor.matmul(
                            ps_o[:],
                            w2_sb[:, i, j * P : (j + 1) * P],
                            hts[i][:],
                            start=(i == 0),
                            stop=(i == HC - 1),
                        )
                    o_t = op_.tile([P, 512], f32, tag="o", name="o_t")[:, :sz]
                    nc.scalar.activation(
                        o_t[:], ps_o[:], AF.Identity, bias=b2_sb[:, j : j + 1]
                    )
                    nc.vector.tensor_tensor(
                        o_t[:], o_t[:], gbc[:, start : start + sz], ALU.mult
                    )
                    nc.sync.dma_start(
                        oT_d.ap()[j * P : (j + 1) * P, start : start + sz], o_t[:]
                    )
    nc.compile()
    return nc


def _build_dense():
    """Fallback: every core computes its expert densely on all B tokens."""
    import concourse.bacc as bacc
    import concourse.bass as bass
    import concourse.mybir as mybir
    import concourse.tile as tile
    from concourse.masks import make_identity

    f32 = mybir.dt.float32
    bf16 = mybir.dt.bfloat16
    AF = mybir.ActivationFunctionType
    ALU = mybir.AluOpType
    AX = mybir.AxisListType

    _patch_act_tables()
    nc = bacc.Bacc("TRN2", target_bir_lowering=False, debug=False)

    xg_d = nc.dram_tensor("xg", [TB, P, KC, P], f32, kind="ExternalInput")
    xb_d = nc.dram_tensor("xb", [NM, P, KC, MC], bf16, kind="ExternalInput")
    w1_d = nc.dram_tensor("w1", [HC, P, D], bf16, kind="ExternalInput")
    w2_d = nc.dram_tensor("w2", [P, HC, O], bf16, kind="ExternalInput")
    b1_d = nc.dram_tensor("b1", [P, HC], f32, kind="ExternalInput")
    b2_d = nc.dram_tensor("b2", [P, OC], f32, kind="ExternalInput")
    wg_d = nc.dram_tensor("wg", [P, KC, 2 * E], f32, kind="ExternalInput")
    nz_d = nc.dram_tensor("nz", [P, TB, E], f32, kind="ExternalInput")
    sel_d = nc.dram_tensor("sel", [1, E], f32, kind="ExternalInput")
    oT_d = nc.dram_tensor("oT", [O, B], f32, kind="ExternalOutput")
    probs_d = nc.dram_tensor("probs", [P, TB * E], f32, kind="ExternalOutput")
    grow_d = nc.dram_tensor("grow", [TB, P], f32)

    with tile.TileContext(nc) as tc:
        with (
            tc.tile_pool(name="persist", bufs=1) as pers,
            tc.tile_pool(name="xgp", bufs=3) as xgp,
            tc.tile_pool(name="xbp", bufs=2) as xbp,
            tc.tile_pool(name="w1p", bufs=4) as w1p,
            tc.tile_pool(name="hp", bufs=36) as hp,
            tc.tile_pool(name="op", bufs=4) as op_,
            tc.tile_pool(name="pg", bufs=2, space="PSUM") as pg,
            tc.tile_pool(name="ptr", bufs=1, space="PSUM") as ptr,
            tc.tile_pool(name="ph", bufs=2, space="PSUM") as ph,
            tc.tile_pool(name="po", bufs=2, space="PSUM") as po,
        ):
            b1_sb = pers.tile([P, HC], f32, tag="b1")
            nc.sync.dma_start(b1_sb[:], b1_d.ap())
            b2_sb = pers.tile([P, OC], f32, tag="b2")
            nc.sync.dma_start(b2_sb[:], b2_d.ap())
            wg_sb = pers.tile([P, KC, 2 * E], f32, tag="wg")
            nc.sync.dma_start(wg_sb[:], wg_d.ap())
            nz_sb = pers.tile([P, TB, E], f32, tag="nz")
            nc.sync.dma_start(nz_sb[:], nz_d.ap())
            sel_sb = pers.tile([P, E], f32, tag="sel")
            nc.gpsimd.dma_start(
                out=sel_sb[:],
                in_=bass.AP(tensor=sel_d, offset=0, ap=[[0, P], [1, E]]),
            )
            ident = pers.tile([P, P], f32, tag="ident")
            make_identity(nc, ident)
            w2_sb = pers.tile([P, HC, O], bf16, tag="w2")
            nc.sync.dma_start(w2_sb[:], w2_d.ap())
            gcol = pers.tile([P, TB], f32, tag="gcol")
            gT_sb = pers.tile([TB, P], f32, tag="gT")
            gbc = pers.tile([P, B], f32, tag="gbc")

            gates_all, probs_sb = _emit_gating(
                nc, tc, bass, mybir, TB, xg_d, wg_sb, nz_sb, (pers, xgp, pg)
            )
            nc.sync.dma_start(probs_d.ap(), probs_sb.rearrange("p t e -> p (t e)"))
            # own-expert gate column: dot(gates, sel)
            gsel = pers.tile([P, TB, E], f32, tag="gsel")
            selb = sel_sb[:, None, :].to_broadcast([P, TB, E])
            nc.vector.tensor_tensor(gsel[:], gates_all[:], selb, ALU.mult)
            nc.vector.tensor_reduce(gcol[:], gsel[:], axis=AX.X, op=ALU.add)
            # broadcast gcol across partitions via transpose + DRAM round-trip
            ps_tr = ptr.tile([TB, P], f32, tag="ptr")
            nc.tensor.transpose(ps_tr[:], gcol[:], ident[:])
            nc.vector.tensor_copy(gT_sb[:], ps_tr[:])
            nc.sync.dma_start(grow_d.ap(), gT_sb[:])
            nc.gpsimd.dma_start(
                out=gbc[:], in_=bass.AP(tensor=grow_d, offset=0, ap=[[0, P], [1, B]])
            )

            for m in range(NM):
                xb_t = xbp.tile([P, KC, MC], bf16, tag="xb")
                nc.sync.dma_start(xb_t[:], xb_d.ap()[m])
                hts = []
                for i in range(HC):
                    w1_t = w1p.tile([P, D], bf16, tag="w1")
                    nc.sync.dma_start(w1_t[:], w1_d.ap()[i])
                    ps_h = ph.tile([P, MC], f32, tag="ph")
                    for c in range(KC):
                        nc.tensor.matmul(
                            ps_h[:],
                            w1_t[:, c * P : (c + 1) * P],
                            xb_t[:, c, :],
                            start=(c == 0),
                            stop=(c == KC - 1),
                        )
                    h_t = hp.tile([P, MC], bf16, tag="h")
                    nc.scalar.activation(
                        h_t[:], ps_h[:], AF.Relu, bias=b1_sb[:, i : i + 1]
                    )
                    hts.append(h_t)
                for j in range(OC):
                    ps_o = po.tile([P, MC], f32, tag="po")
                    for i in range(HC):
                        nc.tensor.matmul(
                            ps_o[:],
                            w2_sb[:, i, j * P : (j + 1) * P],
                            hts[i][:],
                            start=(i == 0),
                            stop=(i == HC - 1),
                        )
                    o_t = op_.tile([P, MC], f32, tag="o")
                    nc.scalar.activation(
                        o_t[:], ps_o[:], AF.Identity, bias=b2_sb[:, j : j + 1]
                    )
                    nc.vector.tensor_tensor(
                        o_t[:], o_t[:], gbc[:, m * MC : (m + 1) * MC], ALU.mult
                    )
                    nc.sync.dma_start(
                        oT_d.ap()[j * P : (j + 1) * P, m * MC : (m + 1) * MC], o_t[:]
                    )
    nc.compile()
    return nc


# ---------------- host side ----------------


def _get(name, builder):
    if name not in _CACHE:
        _CACHE[name] = builder()
    return _CACHE[name]


def _run(nc, in_maps):
    from concourse.bass_utils import run_bass_kernel_spmd

    return run_bass_kernel_spmd(nc, in_maps, core_ids=list(range(NCORE)), trace=False)


def _tile_wg_nz(w_gate, w_noise, noise):
    wgcat = np.concatenate(
        [np.asarray(w_gate, np.float32), np.asarray(w_noise, np.float32)], axis=1
    )
    wg = np.ascontiguousarray(wgcat.reshape(KC, P, 2 * E).transpose(1, 0, 2))
    nz = np.ascontiguousarray(
        np.asarray(noise, np.float32).reshape(B, E).reshape(TB, P, E).transpose(1, 0, 2)
    )
    return wg, nz


def _tile_xg(x):
    return np.ascontiguousarray(x.reshape(TB, P, KC, P).transpose(0, 3, 2, 1))


def _tile_expert(W1, b1, W2, b2, e):
    bf16 = ml_dtypes.bfloat16
    w1e = np.asarray(W1[e], np.float32).astype(bf16)
    w1t = np.ascontiguousarray(
        w1e.reshape(KC, P, HC, P).transpose(2, 1, 0, 3).reshape(HC, P, D)
    )
    w2e = np.asarray(W2[e], np.float32).astype(bf16)
    w2t = np.ascontiguousarray(w2e.reshape(HC, P, O).transpose(1, 0, 2))
    b1t = np.ascontiguousarray(np.asarray(b1[e], np.float32).reshape(HC, P).T)
    b2t = np.ascontiguousarray(np.asarray(b2[e], np.float32).reshape(OC, P).T)
    return w1t, w2t, b1t, b2t


def _finish(y, probs_full, expand_size):
    es = int(np.asarray(expand_size))
    out = np.zeros((B, es, O), np.float32)
    out[:, 0, :] = y.astype(np.float32)
    lf = probs_full.reshape(-1).astype(np.float64)
    loss = LOSS_COEF * np.var(lf, ddof=1) / (np.mean(lf) ** 2 + 1e-10)
    return out, np.float32(loss)


def _kernel_sparse(x, noise, expand_size, w_gate, w_noise, W1, b1, W2, b2):
    bf16 = ml_dtypes.bfloat16
    x = np.ascontiguousarray(np.asarray(x, dtype=np.float32))
    wg, nz = _tile_wg_nz(w_gate, w_noise, noise)
    xg = _tile_xg(x)

    # ---- phase 1: gating, token-parallel ----
    nc1 = _get("gate", _build_gate)
    in1 = []
    for i in range(NCORE):
        in1.append(
            {
                "xg": np.ascontiguousarray(xg[i * TBL : (i + 1) * TBL]),
                "wg": wg,
                "nz": np.ascontiguousarray(nz[:, i * TBL : (i + 1) * TBL, :]),
            }
        )
    r1 = _run(nc1, in1)
    gates_full = np.empty((B, E), np.float32)
    probs_full = np.empty((B, E), np.float32)
    for i in range(NCORE):
        g = r1.results[i]["gates"].reshape(P, TBL, E).transpose(1, 0, 2)
        p = r1.results[i]["probs"].reshape(P, TBL, E).transpose(1, 0, 2)
        gates_full[i * TBL * P : (i + 1) * TBL * P] = g.reshape(TBL * P, E)
        probs_full[i * TBL * P : (i + 1) * TBL * P] = p.reshape(TBL * P, E)

    # ---- host routing ----
    idxs, gvals = [], []
    for e in range(E):
        idx = np.nonzero(gates_full[:, e] > 0.0)[0]
        if len(idx) > CAP:
            return None  # overflow -> dense fallback
        idxs.append(idx)
        gvals.append(gates_full[idx, e])

    x_bf = x.astype(bf16)
    nc2 = _get("mlp", _build_mlp)
    in2 = []
    for e in range(E):
        idx = idxs[e]
        xe = np.zeros((CAP, D), bf16)
        xe[: len(idx)] = x_bf[idx]
        xbt = np.ascontiguousarray(xe.reshape(CAP, KC, P).transpose(2, 1, 0))
        gpad = np.zeros((1, CAP), np.float32)
        gpad[0, : len(idx)] = gvals[e]
        w1t, w2t, b1t, b2t = _tile_expert(W1, b1, W2, b2, e)
        in2.append(
            {
                "xbt": xbt,
                "w1": w1t,
                "w2": w2t,
                "b1": b1t,
                "b2": b2t,
                "g": gpad,
            }
        )
    r2 = _run(nc2, in2)

    y = np.zeros((B, O), np.float64)
    for e in range(E):
        cnt = len(idxs[e])
        y[idxs[e]] += r2.results[e]["oT"][:, :cnt].T.astype(np.float64)
    return _finish(y, probs_full, expand_size)


def _kernel_dense(x, noise, expand_size, w_gate, w_noise, W1, b1, W2, b2):
    bf16 = ml_dtypes.bfloat16
    x = np.ascontiguousarray(np.asarray(x, dtype=np.float32))
    wg, nz = _tile_wg_nz(w_gate, w_noise, noise)
    xg = _tile_xg(x)
    xbf = x.astype(bf16)
    xb = np.ascontiguousarray(xbf.reshape(NM, MC, KC, P).transpose(0, 3, 2, 1))

    nc = _get("dense", _build_dense)
    in_maps = []
    for e in range(E):
        w1t, w2t, b1t, b2t = _tile_expert(W1, b1, W2, b2, e)
        sel = np.zeros((1, E), np.float32)
        sel[0, e] = 1.0
        in_maps.append(
            {
                "xg": xg, "xb": xb, "w1": w1t, "w2": w2t, "b1": b1t,
                "b2": b2t, "wg": wg, "nz": nz, "sel": sel,
            }
        )
    res = _run(nc, in_maps)

    oT_sum = np.zeros((O, B), np.float64)
    for e in range(E):
        oT_sum += res.results[e]["oT"].astype(np.float64)
    y = oT_sum.T
    probs_full = (
        res.results[0]["probs"].reshape(P, TB, E).transpose(1, 0, 2).reshape(B, E)
    )
    return _finish(y, probs_full, expand_size)


def kernel(x, noise, expand_size, w_gate, w_noise, W1, b1, W2, b2):
    r = _kernel_sparse(x, noise, expand_size, w_gate, w_noise, W1, b1, W2, b2)
    if r is None:
        r = _kernel_dense(x, noise, expand_size, w_gate, w_noise, W1, b1, W2, b2)
    return r


# revision 21
# speedup vs baseline: 1.0601x; 1.0188x over previous
"""MoE (noisy top-k gating, E=8 experts, K=4) forward on 8 trn2 NeuronCores.

Sharding: expert-parallel with capacity-based token gathering.

Phase 1 (device, token-parallel): each core computes the noisy-top-k gating
for B/8 tokens (fp32 matmuls + Max8 sort + Exp/Erf activations) and returns
dense gates [B/8, E] and the top-k inclusion probabilities (for the
load-balance loss).

Host routing: from the device-computed gates, build each expert's token
list (~B*K/E tokens), pad to CAP, gather the bf16 token vectors.

Phase 2 (device, expert-parallel): core e runs the dense 2-layer MLP for
its expert over its gathered CAP tokens in bf16 (transposed-activation
dataflow xT -> hT -> oT), scales by the gathered gate row, and returns the
partial oT [O, CAP]. The host scatter-adds the 8 partials into y (the
expert-combine reduction) and finishes the scalar loss.

If any expert is assigned more than CAP tokens (never for the benchmark
shapes: observed max 2101 vs CAP 2304), a dense fallback kernel computes
all 4096 tokens on every expert.
"""

import numpy as np
import ml_dtypes

B, D, H, O, E, K = 4096, 1024, 4096, 1024, 8, 4
NOISE_EPS = 0.01
LOSS_COEF = 0.01

P = 128          # partitions
TB = B // P      # 32 token tiles of 128
KC = D // P      # 8 contraction chunks for D
HC = H // P      # 32 h tiles
OC = O // P      # 8 o tiles

NCORE = 8
TBL = TB // NCORE  # 4 token tiles per core in phase 1

CAP = 2176       # per-expert token capacity (margin over observed max 2101)
# large chunks first: chunk-0's layer-1 span covers the w2 prefetch DMA
CHUNKS = [(0, 512), (512, 512), (1024, 512), (1536, 384), (1920, 256)]

MC = 512         # dense-fallback chunk
NM = B // MC

_CACHE = {}


def _patch_act_tables():
    """Steer Exp/Ln to the combined natural_log_exp table set.

    The act-table chooser greedily picks the first set containing each
    function, bouncing exp_and_others -> natural_log -> exp_and_others ->
    sigmoid (4 x ~2.7us loads) for our Exp,Ln,Exp,Erf sequence. Hiding Exp/Ln
    from the single-function sets makes it settle on natural_log_exp (2
    loads). Only the choice changes; set ids still index act_info.json.
    """
    import concourse.bacc as bacc
    import concourse.mybir as mybir

    if getattr(bacc, "_moe_act_patched", False):
        return
    orig = bacc.get_activation_tables
    AF = mybir.ActivationFunctionType

    def patched(arch):
        t = {k: set(v) for k, v in orig(arch).items()}
        if "natural_log_exp_and_others" in t:
            t.get("exp_and_others", set()).discard(AF.Exp)
            t.get("natural_log", set()).discard(AF.Ln)
        return t

    bacc.get_activation_tables = patched
    bacc._moe_act_patched = True


def _emit_gating(nc, tc, bass, mybir, tbl, xg_d, wg_sb, nz_sb, pools):
    """Gating math for tbl token tiles. Returns (gates_all, probs_sb) sbuf APs."""
    f32 = mybir.dt.float32
    AF = mybir.ActivationFunctionType
    ALU = mybir.AluOpType
    AX = mybir.AxisListType
    pers, xgp, pg = pools

    raw_all = pers.tile([P, tbl, 2 * E], f32, tag="raw")
    sp_all = pers.tile([P, tbl, E], f32, tag="sp")
    noisy_all = pers.tile([P, tbl, E], f32, tag="noisy")
    sort_all = pers.tile([P, tbl, E], f32, tag="sort")
    diff_all = pers.tile([P, tbl, E], f32, tag="diff")
    expd_all = pers.tile([P, tbl, E], f32, tag="expd")
    mask_all = pers.tile([P, tbl, E], f32, tag="mask")
    gme_all = pers.tile([P, tbl, E], f32, tag="gme")
    gs_all = pers.tile([P, tbl], f32, tag="gs")
    rs_all = pers.tile([P, tbl], f32, tag="rs")
    zin_all = pers.tile([P, tbl, E], f32, tag="zin")
    zout_all = pers.tile([P, tbl, E], f32, tag="zout")
    rstd_all = pers.tile([P, tbl, E], f32, tag="rstd")
    min_all = pers.tile([P, tbl, E], mybir.dt.uint32, tag="min")
    pin_all = pers.tile([P, tbl, E], f32, tag="pin")
    probs_sb = pers.tile([P, tbl, E], f32, tag="probs")
    gates_all = pers.tile([P, tbl, E], f32, tag="gates")

    clean_all = raw_all[:, :, 0:E]

    for t in range(tbl):
        xg_t = xgp.tile([P, KC, P], f32, tag="xg")
        if t == 0:
            # per-k split: the first matmul only waits for its own k-slice
            for c in range(KC):
                nc.sync.dma_start(xg_t[:, c, :], xg_d.ap()[t][:, c, :])
        else:
            nc.sync.dma_start(xg_t[:], xg_d.ap()[t])
        ps = pg.tile([P, 2 * E], f32, tag="pg")
        for c in range(KC):
            nc.tensor.matmul(
                ps[:], xg_t[:, c, :], wg_sb[:, c, :],
                start=(c == 0), stop=(c == KC - 1),
            )
        nc.vector.tensor_copy(raw_all[:, t, :], ps[:])

    # stddev = softplus(rawnoise) + eps = ln(1 + exp(r)) + eps
    nc.scalar.activation(sp_all[:], raw_all[:, :, E : 2 * E], AF.Exp)
    nc.scalar.activation(sp_all[:], sp_all[:], AF.Ln, bias=1.0)
    nc.vector.tensor_scalar_add(sp_all[:], sp_all[:], NOISE_EPS)
    # noisy = clean + noise * stddev
    nc.vector.tensor_tensor(noisy_all[:], nz_sb[:], sp_all[:], ALU.mult)
    nc.vector.tensor_tensor(noisy_all[:], noisy_all[:], clean_all, ALU.add)
    nc.vector.reciprocal(rstd_all[:], sp_all[:])
    for t in range(tbl):
        nc.vector.max(sort_all[:, t, :], noisy_all[:, t, :])
    # batched threshold ops: broadcast the kth/(k+1)th values along E
    thr4b = sort_all[:, :, K - 1 : K].to_broadcast([P, tbl, E])
    thr5b = sort_all[:, :, K : K + 1].to_broadcast([P, tbl, E])
    nc.vector.tensor_tensor(diff_all[:], noisy_all[:], thr4b, ALU.subtract)
    nc.vector.tensor_tensor(zin_all[:], clean_all, thr5b, ALU.subtract)
    nc.vector.tensor_tensor(zout_all[:], clean_all, thr4b, ALU.subtract)
    nc.vector.tensor_tensor(min_all[:], noisy_all[:], thr5b, ALU.is_gt)
    nc.scalar.activation(expd_all[:], diff_all[:], AF.Exp)
    nc.vector.tensor_scalar(mask_all[:], diff_all[:], 0.0, None, op0=ALU.is_ge)
    nc.vector.tensor_tensor(gme_all[:], expd_all[:], mask_all[:], ALU.mult)
    nc.vector.tensor_reduce(gs_all[:], gme_all[:], axis=AX.X, op=ALU.add)
    nc.vector.reciprocal(rs_all[:], gs_all[:])
    rsb = rs_all[:, :, None].to_broadcast([P, tbl, E])
    nc.vector.tensor_tensor(gates_all[:], gme_all[:], rsb, ALU.mult)
    # prob = Phi(z) = 0.5 * erf(z / sqrt(2)) + 0.5
    nc.vector.tensor_tensor(zin_all[:], zin_all[:], rstd_all[:], ALU.mult)
    nc.vector.tensor_tensor(zout_all[:], zout_all[:], rstd_all[:], ALU.mult)
    isq2 = float(1.0 / np.sqrt(2.0))
    nc.scalar.activation(pin_all[:], zin_all[:], AF.Erf, scale=isq2)
    nc.scalar.activation(probs_sb[:], zout_all[:], AF.Erf, scale=isq2)
    nc.vector.tensor_scalar(
        pin_all[:], pin_all[:], 0.5, 0.5, op0=ALU.mult, op1=ALU.add
    )
    nc.vector.tensor_scalar(
        probs_sb[:], probs_sb[:], 0.5, 0.5, op0=ALU.mult, op1=ALU.add
    )
    nc.vector.copy_predicated(probs_sb[:], min_all[:], pin_all[:])
    return gates_all, probs_sb


def _build_gate():
    """Phase-1: token-parallel gating; each core handles B/8 tokens."""
    import concourse.bacc as bacc
    import concourse.bass as bass
    import concourse.mybir as mybir
    import concourse.tile as tile

    _patch_act_tables()
    f32 = mybir.dt.float32
    nc = bacc.Bacc("TRN2", target_bir_lowering=False, debug=False)

    xg_d = nc.dram_tensor("xg", [TBL, P, KC, P], f32, kind="ExternalInput")
    wg_d = nc.dram_tensor("wg", [P, KC, 2 * E], f32, kind="ExternalInput")
    nz_d = nc.dram_tensor("nz", [P, TBL, E], f32, kind="ExternalInput")
    gates_d = nc.dram_tensor("gates", [P, TBL * E], f32, kind="ExternalOutput")
    probs_d = nc.dram_tensor("probs", [P, TBL * E], f32, kind="ExternalOutput")

    with tile.TileContext(nc) as tc:
        with (
            tc.tile_pool(name="persist", bufs=1) as pers,
            tc.tile_pool(name="xgp", bufs=3) as xgp,
            tc.tile_pool(name="pg", bufs=2, space="PSUM") as pg,
        ):
            wg_sb = pers.tile([P, KC, 2 * E], f32, tag="wg")
            nc.sync.dma_start(wg_sb[:], wg_d.ap())
            nz_sb = pers.tile([P, TBL, E], f32, tag="nz")
            nc.sync.dma_start(nz_sb[:], nz_d.ap())
            gates_all, probs_sb = _emit_gating(
                nc, tc, bass, mybir, TBL, xg_d, wg_sb, nz_sb, (pers, xgp, pg)
            )
            nc.sync.dma_start(gates_d.ap(), gates_all.rearrange("p t e -> p (t e)"))
            nc.sync.dma_start(probs_d.ap(), probs_sb.rearrange("p t e -> p (t e)"))
    nc.compile()
    return nc


def _build_mlp():
    """Phase-2: expert-parallel MLP over CAP gathered tokens per core."""
    import concourse.bacc as bacc
    import concourse.bass as bass
    import concourse.mybir as mybir
    import concourse.tile as tile

    f32 = mybir.dt.float32
    bf16 = mybir.dt.bfloat16
    AF = mybir.ActivationFunctionType
    ALU = mybir.AluOpType

    _patch_act_tables()
    nc = bacc.Bacc("TRN2", target_bir_lowering=False, debug=False)

    xbt_d = nc.dram_tensor("xbt", [P, KC, CAP], bf16, kind="ExternalInput")
    w1_d = nc.dram_tensor("w1", [HC, P, D], bf16, kind="ExternalInput")
    w2_d = nc.dram_tensor("w2", [P, HC, O], bf16, kind="ExternalInput")
    b1_d = nc.dram_tensor("b1", [P, HC], f32, kind="ExternalInput")
    b2_d = nc.dram_tensor("b2", [P, OC], f32, kind="ExternalInput")
    g_d = nc.dram_tensor("g", [1, CAP], f32, kind="ExternalInput")
    oT_d = nc.dram_tensor("oT", [O, CAP], f32, kind="ExternalOutput")

    with tile.TileContext(nc) as tc:
        with (
            tc.tile_pool(name="persist", bufs=1) as pers,
            tc.tile_pool(name="xbp", bufs=2) as xbp,
            tc.tile_pool(name="w1p", bufs=4) as w1p,
            tc.tile_pool(name="hp", bufs=36) as hp,
            tc.tile_pool(name="op", bufs=4) as op_,
            tc.tile_pool(name="ph", bufs=3, space="PSUM") as ph,
            tc.tile_pool(name="po", bufs=3, space="PSUM") as po,
        ):
            b1_sb = pers.tile([P, HC], f32, tag="b1")
            nc.sync.dma_start(b1_sb[:], b1_d.ap())
            b2_sb = pers.tile([P, OC], f32, tag="b2")
            nc.sync.dma_start(b2_sb[:], b2_d.ap())
            gbc = pers.tile([P, CAP], f32, tag="gbc")
            nc.gpsimd.dma_start(
                out=gbc[:], in_=bass.AP(tensor=g_d, offset=0, ap=[[0, P], [1, CAP]])
            )
            # w2 is DMA'd in quarters, emitted after the first chunk's layer-1
            # so the startup DMAs that gate the first matmuls go first.
            w2_sb = pers.tile([P, HC, O], bf16, tag="w2")
            w2_started = False
            # first HC/2 w1 tiles stay resident: halves the per-chunk w1
            # streaming rate, which the narrow chunks cannot otherwise sustain
            HR = HC // 2
            w1r = pers.tile([P, HR, D], bf16, tag="w1r")

            for ci, (start, sz) in enumerate(CHUNKS):
                xb_t = xbp.tile([P, KC, 512], bf16, tag="xb", name="xb_t")[:, :, :sz]
                # issue order drives the DMA queue: the first matmul's inputs
                # (w1 tile 0 first half, xb k-slice 0) must be issued first
                if ci == 0:
                    nc.sync.dma_start(w1r[:, 0, : D // 2], w1_d.ap()[0][:, : D // 2])
                nc.sync.dma_start(xb_t[:, 0, :], xbt_d.ap()[:, 0, start : start + sz])
                if ci == 0:
                    nc.sync.dma_start(w1r[:, 0, D // 2 :], w1_d.ap()[0][:, D // 2 :])
                for c in range(1, KC):
                    nc.sync.dma_start(
                        xb_t[:, c, :], xbt_d.ap()[:, c, start : start + sz]
                    )
                hts = []
                for i in range(HC):
                    if i < HR:
                        w1_t = w1r[:, i, :]
                        if ci == 0 and i > 0:
                            nc.sync.dma_start(
                                w1_t[:, : D // 2], w1_d.ap()[i][:, : D // 2]
                            )
                            nc.sync.dma_start(
                                w1_t[:, D // 2 :], w1_d.ap()[i][:, D // 2 :]
                            )
                    else:
                        w1_t = w1p.tile([P, D], bf16, tag="w1")
                        nc.sync.dma_start(
                            w1_t[:, : D // 2], w1_d.ap()[i][:, : D // 2]
                        )
                        nc.sync.dma_start(
                            w1_t[:, D // 2 :], w1_d.ap()[i][:, D // 2 :]
                        )
                    ps_h = ph.tile([P, 512], f32, tag="ph", name="ps_h")[:, :sz]
                    for c in range(KC):
                        nc.tensor.matmul(
                            ps_h[:],
                            w1_t[:, c * P : (c + 1) * P],
                            xb_t[:, c, :],
                            start=(c == 0),
                            stop=(c == KC - 1),
                        )
                    h_t = hp.tile([P, 512], bf16, tag="h", name="h_t")[:, :sz]
                    nc.scalar.activation(
                        h_t[:], ps_h[:], AF.Relu, bias=b1_sb[:, i : i + 1]
                    )
                    hts.append(h_t)
                    # interleave the w2 prefetch quarters into chunk-0's
                    # layer-1 so layer-2 never waits on them
                    if ci == 0 and i >= HR and (i - HR) % 4 == 0:
                        q = HC // 4
                        qi = (i - HR) // 4
                        nc.sync.dma_start(
                            w2_sb[:, qi * q : (qi + 1) * q, :],
                            w2_d.ap()[:, qi * q : (qi + 1) * q, :],
                        )
                for j in range(OC):
                    ps_o = po.tile([P, 512], f32, tag="po", name="ps_o")[:, :sz]
                    for i in range(HC):
                        nc.tensor.matmul(
                            ps_o[:],
                            w2_sb[:, i, j * P : (j + 1) * P],
                            hts[i][:],
                            start=(i == 0),
                            stop=(i == HC - 1),
                        )
                    o_t = op_.tile([P, 512], f32, tag="o", name="o_t")[:, :sz]
                    nc.scalar.activation(
                        o_t[:], ps_o[:], AF.Identity, bias=b2_sb[:, j : j + 1]
                    )
                    nc.vector.tensor_tensor(
                        o_t[:], o_t[:], gbc[:, start : start + sz], ALU.mult
                    )
                    nc.sync.dma_start(
                        oT_d.ap()[j * P : (j + 1) * P, start : start + sz], o_t[:]
                    )
    nc.compile()
    return nc


def _build_dense():
    """Fallback: every core computes its expert densely on all B tokens."""
    import concourse.bacc as bacc
    import concourse.bass as bass
    import concourse.mybir as mybir
    import concourse.tile as tile
    from concourse.masks import make_identity

    f32 = mybir.dt.float32
    bf16 = mybir.dt.bfloat16
    AF = mybir.ActivationFunctionType
    ALU = mybir.AluOpType
    AX = mybir.AxisListType

    _patch_act_tables()
    nc = bacc.Bacc("TRN2", target_bir_lowering=False, debug=False)

    xg_d = nc.dram_tensor("xg", [TB, P, KC, P], f32, kind="ExternalInput")
    xb_d = nc.dram_tensor("xb", [NM, P, KC, MC], bf16, kind="ExternalInput")
    w1_d = nc.dram_tensor("w1", [HC, P, D], bf16, kind="ExternalInput")
    w2_d = nc.dram_tensor("w2", [P, HC, O], bf16, kind="ExternalInput")
    b1_d = nc.dram_tensor("b1", [P, HC], f32, kind="ExternalInput")
    b2_d = nc.dram_tensor("b2", [P, OC], f32, kind="ExternalInput")
    wg_d = nc.dram_tensor("wg", [P, KC, 2 * E], f32, kind="ExternalInput")
    nz_d = nc.dram_tensor("nz", [P, TB, E], f32, kind="ExternalInput")
    sel_d = nc.dram_tensor("sel", [1, E], f32, kind="ExternalInput")
    oT_d = nc.dram_tensor("oT", [O, B], f32, kind="ExternalOutput")
    probs_d = nc.dram_tensor("probs", [P, TB * E], f32, kind="ExternalOutput")
    grow_d = nc.dram_tensor("grow", [TB, P], f32)

    with tile.TileContext(nc) as tc:
        with (
            tc.tile_pool(name="persist", bufs=1) as pers,
            tc.tile_pool(name="xgp", bufs=3) as xgp,
            tc.tile_pool(name="xbp", bufs=2) as xbp,
            tc.tile_pool(name="w1p", bufs=4) as w1p,
            tc.tile_pool(name="hp", bufs=36) as hp,
            tc.tile_pool(name="op", bufs=4) as op_,
            tc.tile_pool(name="pg", bufs=2, space="PSUM") as pg,
            tc.tile_pool(name="ptr", bufs=1, space="PSUM") as ptr,
            tc.tile_pool(name="ph", bufs=2, space="PSUM") as ph,
            tc.tile_pool(name="po", bufs=2, space="PSUM") as po,
        ):
            b1_sb = pers.tile([P, HC], f32, tag="b1")
            nc.sync.dma_start(b1_sb[:], b1_d.ap())
            b2_sb = pers.tile([P, OC], f32, tag="b2")
            nc.sync.dma_start(b2_sb[:], b2_d.ap())
            wg_sb = pers.tile([P, KC, 2 * E], f32, tag="wg")
            nc.sync.dma_start(wg_sb[:], wg_d.ap())
            nz_sb = pers.tile([P, TB, E], f32, tag="nz")
            nc.sync.dma_start(nz_sb[:], nz_d.ap())
            sel_sb = pers.tile([P, E], f32, tag="sel")
            nc.gpsimd.dma_start(
                out=sel_sb[:],
                in_=bass.AP(tensor=sel_d, offset=0, ap=[[0, P], [1, E]]),
            )
            ident = pers.tile([P, P], f32, tag="ident")
            make_identity(nc, ident)
            w2_sb = pers.tile([P, HC, O], bf16, tag="w2")
            nc.sync.dma_start(w2_sb[:], w2_d.ap())
            gcol = pers.tile([P, TB], f32, tag="gcol")
            gT_sb = pers.tile([TB, P], f32, tag="gT")
            gbc = pers.tile([P, B], f32, tag="gbc")

            gates_all, probs_sb = _emit_gating(
                nc, tc, bass, mybir, TB, xg_d, wg_sb, nz_sb, (pers, xgp, pg)
            )
            nc.sync.dma_start(probs_d.ap(), probs_sb.rearrange("p t e -> p (t e)"))
            # own-expert gate column: dot(gates, sel)
            gsel = pers.tile([P, TB, E], f32, tag="gsel")
            selb = sel_sb[:, None, :].to_broadcast([P, TB, E])
            nc.vector.tensor_tensor(gsel[:], gates_all[:], selb, ALU.mult)
            nc.vector.tensor_reduce(gcol[:], gsel[:], axis=AX.X, op=ALU.add)
            # broadcast gcol across partitions via transpose + DRAM round-trip
            ps_tr = ptr.tile([TB, P], f32, tag="ptr")
            nc.tensor.transpose(ps_tr[:], gcol[:], ident[:])
            nc.vector.tensor_copy(gT_sb[:], ps_tr[:])
            nc.sync.dma_start(grow_d.ap(), gT_sb[:])
            nc.gpsimd.dma_start(
                out=gbc[:], in_=bass.AP(tensor=grow_d, offset=0, ap=[[0, P], [1, B]])
            )

            for m in range(NM):
                xb_t = xbp.tile([P, KC, MC], bf16, tag="xb")
                nc.sync.dma_start(xb_t[:], xb_d.ap()[m])
                hts = []
                for i in range(HC):
                    w1_t = w1p.tile([P, D], bf16, tag="w1")
                    nc.sync.dma_start(w1_t[:], w1_d.ap()[i])
                    ps_h = ph.tile([P, MC], f32, tag="ph")
                    for c in range(KC):
                        nc.tensor.matmul(
                            ps_h[:],
                            w1_t[:, c * P : (c + 1) * P],
                            xb_t[:, c, :],
                            start=(c == 0),
                            stop=(c == KC - 1),
                        )
                    h_t = hp.tile([P, MC], bf16, tag="h")
                    nc.scalar.activation(
                        h_t[:], ps_h[:], AF.Relu, bias=b1_sb[:, i : i + 1]
                    )
                    hts.append(h_t)
                for j in range(OC):
                    ps_o = po.tile([P, MC], f32, tag="po")
                    for i in range(HC):
                        nc.tensor.matmul(
                            ps_o[:],
                            w2_sb[:, i, j * P : (j + 1) * P],
                            hts[i][:],
                            start=(i == 0),
                            stop=(i == HC - 1),
                        )
                    o_t = op_.tile([P, MC], f32, tag="o")
                    nc.scalar.activation(
                        o_t[:], ps_o[:], AF.Identity, bias=b2_sb[:, j : j + 1]
                    )
                    nc.vector.tensor_tensor(
                        o_t[:], o_t[:], gbc[:, m * MC : (m + 1) * MC], ALU.mult
                    )
                    nc.sync.dma_start(
                        oT_d.ap()[j * P : (j + 1) * P, m * MC : (m + 1) * MC], o_t[:]
                    )
    nc.compile()
    return nc


# ---------------- host side ----------------


def _get(name, builder):
    if name not in _CACHE:
        _CACHE[name] = builder()
    return _CACHE[name]


def _run(nc, in_maps):
    from concourse.bass_utils import run_bass_kernel_spmd

    return run_bass_kernel_spmd(nc, in_maps, core_ids=list(range(NCORE)), trace=False)


def _tile_wg_nz(w_gate, w_noise, noise):
    wgcat = np.concatenate(
        [np.asarray(w_gate, np.float32), np.asarray(w_noise, np.float32)], axis=1
    )
    wg = np.ascontiguousarray(wgcat.reshape(KC, P, 2 * E).transpose(1, 0, 2))
    nz = np.ascontiguousarray(
        np.asarray(noise, np.float32).reshape(B, E).reshape(TB, P, E).transpose(1, 0, 2)
    )
    return wg, nz


def _tile_xg(x):
    return np.ascontiguousarray(x.reshape(TB, P, KC, P).transpose(0, 3, 2, 1))


def _tile_expert(W1, b1, W2, b2, e):
    bf16 = ml_dtypes.bfloat16
    w1e = np.asarray(W1[e], np.float32).astype(bf16)
    w1t = np.ascontiguousarray(
        w1e.reshape(KC, P, HC, P).transpose(2, 1, 0, 3).reshape(HC, P, D)
    )
    w2e = np.asarray(W2[e], np.float32).astype(bf16)
    w2t = np.ascontiguousarray(w2e.reshape(HC, P, O).transpose(1, 0, 2))
    b1t = np.ascontiguousarray(np.asarray(b1[e], np.float32).reshape(HC, P).T)
    b2t = np.ascontiguousarray(np.asarray(b2[e], np.float32).reshape(OC, P).T)
    return w1t, w2t, b1t, b2t


def _finish(y, probs_full, expand_size):
    es = int(np.asarray(expand_size))
    out = np.zeros((B, es, O), np.float32)
    out[:, 0, :] = y.astype(np.float32)
    lf = probs_full.reshape(-1).astype(np.float64)
    loss = LOSS_COEF * np.var(lf, ddof=1) / (np.mean(lf) ** 2 + 1e-10)
    return out, np.float32(loss)


def _kernel_sparse(x, noise, expand_size, w_gate, w_noise, W1, b1, W2, b2):
    bf16 = ml_dtypes.bfloat16
    x = np.ascontiguousarray(np.asarray(x, dtype=np.float32))
    wg, nz = _tile_wg_nz(w_gate, w_noise, noise)
    xg = _tile_xg(x)

    # ---- phase 1: gating, token-parallel ----
    nc1 = _get("gate", _build_gate)
    in1 = []
    for i in range(NCORE):
        in1.append(
            {
                "xg": np.ascontiguousarray(xg[i * TBL : (i + 1) * TBL]),
                "wg": wg,
                "nz": np.ascontiguousarray(nz[:, i * TBL : (i + 1) * TBL, :]),
            }
        )
    r1 = _run(nc1, in1)
    gates_full = np.empty((B, E), np.float32)
    probs_full = np.empty((B, E), np.float32)
    for i in range(NCORE):
        g = r1.results[i]["gates"].reshape(P, TBL, E).transpose(1, 0, 2)
        p = r1.results[i]["probs"].reshape(P, TBL, E).transpose(1, 0, 2)
        gates_full[i * TBL * P : (i + 1) * TBL * P] = g.reshape(TBL * P, E)
        probs_full[i * TBL * P : (i + 1) * TBL * P] = p.reshape(TBL * P, E)

    # ---- host routing ----
    idxs, gvals = [], []
    for e in range(E):
        idx = np.nonzero(gates_full[:, e] > 0.0)[0]
        if len(idx) > CAP:
            return None  # overflow -> dense fallback
        idxs.append(idx)
        gvals.append(gates_full[idx, e])

    x_bf = x.astype(bf16)
    nc2 = _get("mlp", _build_mlp)
    in2 = []
    for e in range(E):
        idx = idxs[e]
        xe = np.zeros((CAP, D), bf16)
        xe[: len(idx)] = x_bf[idx]
        xbt = np.ascontiguousarray(xe.reshape(CAP, KC, P).transpose(2, 1, 0))
        gpad = np.zeros((1, CAP), np.float32)
        gpad[0, : len(idx)] = gvals[e]
        w1t, w2t, b1t, b2t = _tile_expert(W1, b1, W2, b2, e)
        in2.append(
            {
                "xbt": xbt,
                "w1": w1t,
                "w2": w2t,
                "b1": b1t,
                "b2": b2t,
                "g": gpad,
            }
        )
    r2 = _run(nc2, in2)

    y = np.zeros((B, O), np.float64)
    for e in range(E):
        cnt = len(idxs[e])
        y[idxs[e]] += r2.results[e]["oT"][:, :cnt].T.astype(np.float64)
    return _finish(y, probs_full, expand_size)


def _kernel_dense(x, noise, expand_size, w_gate, w_noise, W1, b1, W2, b2):
    bf16 = ml_dtypes.bfloat16
    x = np.ascontiguousarray(np.asarray(x, dtype=np.float32))
    wg, nz = _tile_wg_nz(w_gate, w_noise, noise)
    xg = _tile_xg(x)
    xbf = x.astype(bf16)
    xb = np.ascontiguousarray(xbf.reshape(NM, MC, KC, P).transpose(0, 3, 2, 1))

    nc = _get("dense", _build_dense)
    in_maps = []
    for e in range(E):
        w1t, w2t, b1t, b2t = _tile_expert(W1, b1, W2, b2, e)
        sel = np.zeros((1, E), np.float32)
        sel[0, e] = 1.0
        in_maps.append(
            {
                "xg": xg, "xb": xb, "w1": w1t, "w2": w2t, "b1": b1t,
                "b2": b2t, "wg": wg, "nz": nz, "sel": sel,
            }
        )
    res = _run(nc, in_maps)

    oT_sum = np.zeros((O, B), np.float64)
    for e in range(E):
        oT_sum += res.results[e]["oT"].astype(np.float64)
    y = oT_sum.T
    probs_full = (
        res.results[0]["probs"].reshape(P, TB, E).transpose(1, 0, 2).reshape(B, E)
    )
    return _finish(y, probs_full, expand_size)


def kernel(x, noise, expand_size, w_gate, w_noise, W1, b1, W2, b2):
    r = _kernel_sparse(x, noise, expand_size, w_gate, w_noise, W1, b1, W2, b2)
    if r is None:
        r = _kernel_dense(x, noise, expand_size, w_gate, w_noise, W1, b1, W2, b2)
    return r
